# revision 52
# baseline (speedup 1.0000x reference)
"""Trainium2 Bass kernel for 3-layer GraphSAGE (nn_MCHCGraphSage).

Strategy (8 NeuronCores, SPMD single program):
  - Destination-sharded edges: core k owns dst nodes [k*6250, (k+1)*6250).
  - Features live in HBM as 256B rows in "split-slab address" space
    (_addr): each core's slab is stored [windows 0..39 | 22 zero pad rows |
    windows 40..48] so the inter-layer AllGather goes out in two contiguous
    pieces — piece A (rows [0,5142)) fires mid-layer and overlaps the tail
    windows, only piece B (1152 rows) sits on the layer boundary. hext is
    double-buffered (hext0/hext1) so a piece-A write never races the
    previous layer's in-flight gathers.
  - Random x[src] rows are fetched with gpsimd dma_gather (int16 indices)
    spread round-robin over 4 SWDGE queues (the aggregate random-256B
    packet rate ~4 ns/packet is the kernel's bottleneck). int16 range
    forces a two-section split: section A gathers rows [0, 32768),
    section B rows [BBASE, TOTROW) (base offset BBASE).
  - Segmented mean via two PE matmul levels over dst-sorted, degree-padded
    (multiple of 4) edge slots:
      level 1: 8 wide matmuls per window (constant block-ones lhsT
               [128, 32], one per PE row-quadrant x section) over
               run-contiguous chunk slices; never-written PSUM cells are
               zero-filled once (first psA-pool rotation) and stay zero.
      level 2: host-built one-hot [128 groups, 128 dst] (bf16) with the
               1/deg mean scale folded into its values, accumulated in
               PSUM; one PSUM->SBUF bf16 cast per window (ACT/DVE
               alternating).
  - Dense part per window, node-major: y = meanT.T @ Wl + hselfT.T @ Ws_ext
    (bias folded as an extra ones-row of hselfT, zeroed on pad columns so
    pad slab rows compute to exactly 0), ReLU on ACT, DMA the [128, 64]
    node-major block straight to the own slab; PE-transpose to keep the
    feature-major self slab for the next layer.
"""

import os
import sys

import numpy as np

for _p in ("/opt/trn_rl_repo", "/root/.axon_site/_ro/trn_rl_repo"):
    if os.path.isdir(_p) and _p not in sys.path:
        sys.path.append(_p)

import ml_dtypes  # noqa: E402

N = 50000
D = 64
NCORES = 8
SLAB = 6250
PSLAB = 6272
WIN = 128
NW = PSLAB // WIN  # 49
# Split-slab layout: the per-core slab is stored as
#   [windows 0..SPLIT_W-1 (H1 rows) | 22 zero pad rows | windows SPLIT_W..48]
# so the inter-layer AllGather can go out in two contiguous pieces: piece A
# (rows [0, H1P)) fires once windows < SPLIT_W are done and overlaps the
# tail windows' compute; only piece B sits on the layer boundary.
SPLIT_W = 40
H1 = SPLIT_W * WIN  # 5120
PADN = PSLAB - SLAB  # 22
H1P = H1 + PADN  # 5142 (piece-A rows per core, incl. always-zero pad)
H2 = PSLAB - H1  # 1152 (piece-B rows per core)
PSLAB2 = H1P + H2  # 6294 stored slab rows per core
TOTROW = NCORES * PSLAB2  # 50352
BASE_B = NCORES * H1P  # 41136, start of piece-B region in hext
BBASE = TOTROW - 32768  # 17584, B-section base row
APAD_ROW = H1  # row 5120 (core 0 piece-A pad) is always zero
BPAD_ROW = BASE_B + (SLAB - H1)  # core 0's s=6250 pad row, always zero
BW = 4  # windows per gather batch

_NC_CACHE = {}
LAST_RESULTS = None  # test harness introspection (exec_time_ns, profile)


def _addr(n):
    s = n % SLAB
    k = n // SLAB
    return np.where(s < H1, k * H1P + s, BASE_B + k * H2 + (s - H1))


def _srow(w):
    """Stored slab row of window w's first node."""
    return w * WIN if w < SPLIT_W else H1P + (w - SPLIT_W) * WIN


def _run_split(nch_a, nch_b):
    """Assign the NCH chunks of a window to 4 PE row-quadrants in
    contiguous runs: A chunks split [nA0..nA3], then B chunks [nB0..nB3].
    Quadrant r holds A-run r at col-blocks [0, nA[r]) and B-run r at
    [nA[r], nA[r]+nB[r])."""
    nA = np.array([(nch_a + 3 - r) // 4 for r in range(4)])
    nB = np.array([(nch_b + 3 - r) // 4 for r in range(4)])
    aoff = np.concatenate([[0], np.cumsum(nA)]).astype(np.int64)
    boff = np.concatenate([[0], np.cumsum(nB)]).astype(np.int64)
    runmapA = np.repeat(np.arange(4), nA)
    runmapB = np.repeat(np.arange(4), nB)
    nblk = max(a + b for a, b in zip(nA, nB))
    return nA, nB, aoff, boff, runmapA, runmapB, nblk


def _pack(x, edge_index, scale, use_bf16):
    """Host-side packing. Returns per-core dicts + structure constants."""
    src = np.asarray(edge_index[0], dtype=np.int64)
    dst = np.asarray(edge_index[1], dtype=np.int64)
    addr_e = _addr(src)

    # pass 1: global section sizes
    nch_a = 0
    nch_b = 0
    per_core = []
    for k in range(NCORES):
        sel = (dst >= k * SLAB) & (dst < (k + 1) * SLAB)
        s_k = src[sel]
        d_k = dst[sel] - k * SLAB
        a_k = addr_e[sel]
        isA = a_k <= 32767
        degA = np.bincount(d_k[isA], minlength=PSLAB)
        degB = np.bincount(d_k[~isA], minlength=PSLAB)
        padA = ((degA + 3) // 4) * 4
        padB = ((degB + 3) // 4) * 4
        wA = padA.reshape(NW, WIN).sum(1).max()
        wB = padB.reshape(NW, WIN).sum(1).max()
        nch_a = max(nch_a, (int(wA) + 127) // 128)
        nch_b = max(nch_b, (int(wB) + 127) // 128)
        per_core.append((d_k, a_k, isA, padA, padB))

    S_A = nch_a * 128
    S_B = nch_b * 128
    nA, nB, aoff, boff, runmapA, runmapB, NBLK = _run_split(nch_a, nch_b)
    fdt = ml_dtypes.bfloat16 if use_bf16 else np.float32
    ROW = 128 if use_bf16 else 64

    # xext: node features in padded-slab address space, same for all cores
    xext = np.zeros((TOTROW, ROW), dtype=fdt)
    rows = _addr(np.arange(N))
    xext[rows, :D] = x.astype(fdt)

    cores = []
    for k in range(NCORES):
        d_k, a_k, isA, padA, padB = per_core[k]
        pA2 = padA.reshape(NW, WIN)
        pB2 = padB.reshape(NW, WIN)
        offA = (np.cumsum(pA2, 1) - pA2).reshape(-1)  # per local dst
        offB = (np.cumsum(pB2, 1) - pB2).reshape(-1)

        def build_stream(mask, off, S, base, padval):
            e_d = d_k[mask]
            e_a = a_k[mask]
            order = np.argsort(e_d, kind="stable")
            d_s = e_d[order]
            a_s = e_a[order]
            deg = np.bincount(e_d, minlength=PSLAB)
            start = np.concatenate([[0], np.cumsum(deg)])[:-1]
            rank = np.arange(len(d_s)) - start[d_s]
            pos = (d_s // WIN) * S + off[d_s] + rank
            stream = np.full(NW * S, padval, dtype=np.int64)
            stream[pos] = a_s - base
            return stream

        streamA = build_stream(isA, offA, S_A, 0, APAD_ROW)
        streamB = build_stream(~isA, offB, S_B, BBASE, BPAD_ROW - BBASE)
        assert streamA.max() <= 32767 and streamB.max() <= 32767
        assert streamA.min() >= 0 and streamB.min() >= 0

        # group -> (partition, col-block) map with 1/deg folded into the
        # one-hot. Chunks are assigned to PE row-quadrants in contiguous
        # runs (A chunks split [nA0..nA3], B chunks [nB0..nB3]) so level-1
        # can be 8 wide matmuls per window over contiguous chunk slices.
        onehot = np.zeros((128, NW * NBLK * 128), dtype=ml_dtypes.bfloat16)

        def add_section(pad, off, runmap, roff, cb_of):
            reps = pad // 4
            tot = int(reps.sum())
            if tot == 0:
                return
            dstrep = np.repeat(np.arange(PSLAB), reps)
            cum = np.cumsum(reps) - reps
            within = np.arange(tot) - np.repeat(cum, reps)
            gsec = (off // 4)[dstrep] + within  # group idx in window-section
            ca = gsec // 32  # chunk within window-section
            gin = gsec % 32  # group within chunk
            r = runmap[ca]
            p = ca - roff[r]  # position within the quadrant's run
            part = 32 * r + gin
            wnum = dstrep // WIN
            cb = cb_of(r, p)
            cols = (wnum * NBLK + cb) * 128 + dstrep % WIN
            onehot[part, cols] = scale[k * SLAB + dstrep]

        add_section(padA, offA, runmapA, aoff, lambda r, p: p)
        if nch_b > 0:
            add_section(padB, offB, runmapB, boff, lambda r, p: nA[r] + p)

        stream = np.concatenate([streamA, streamB]).astype(np.int16)
        idx16 = stream.reshape(-1, 16).T.copy()  # [16, T/16]
        idx = np.tile(idx16, (8, 1))  # replicate for 8 gpsimd cores

        xselfT = np.zeros((D + 1, PSLAB), dtype=fdt)
        xselfT[:D, :SLAB] = x[k * SLAB : (k + 1) * SLAB].T.astype(fdt)
        xselfT[D, :SLAB] = 1.0  # bias row; pad columns stay 0 -> relu(0)=0

        cores.append({"idx": idx, "onehot": onehot, "xselfT": xselfT})

    return nch_a, nch_b, NBLK, xext, cores


def _build_nc(nch_a, nch_b, nblk, use_bf16):
    import concourse.bacc as bacc
    import concourse.tile as tile
    import concourse.mybir as mybir

    dt = mybir.dt
    fdt = dt.bfloat16 if use_bf16 else dt.float32
    ROW = 128 if use_bf16 else 64
    NCH = nch_a + nch_b
    S_A = nch_a * 128
    S_B = nch_b * 128
    T_A = NW * S_A
    T_B = NW * S_B
    nA, nB, aoff, boff, _, _, nblk_chk = _run_split(nch_a, nch_b)
    assert nblk_chk == nblk
    max_fill = max(nblk - a - b for a, b in zip(nA, nB))

    nqueues = int(os.environ.get("SAGE_QUEUES", "4"))
    use_prep = os.environ.get("SAGE_PREP", "") == "1"
    nc = bacc.Bacc(None, num_devices=NCORES, num_swdge_queues=nqueues)

    xext_d = nc.dram_tensor("xext", [TOTROW, ROW], fdt, kind="ExternalInput")
    idx_d = nc.dram_tensor(
        "idx", [128, (T_A + T_B) // 16], dt.int16, kind="ExternalInput"
    )
    oh_d = nc.dram_tensor(
        "onehot", [128, NW * nblk * 128], dt.bfloat16, kind="ExternalInput"
    )
    xsT_d = nc.dram_tensor("xselfT", [D + 1, PSLAB], fdt, kind="ExternalInput")
    bones_d = nc.dram_tensor("bones", [128, 32], fdt, kind="ExternalInput")
    ident_d = nc.dram_tensor("ident", [WIN, WIN], fdt, kind="ExternalInput")
    w_d = {}
    for l, m in ((0, D), (1, D), (2, 1)):
        w_d[f"wl{l}"] = nc.dram_tensor(f"wl{l}", [D, m], fdt, kind="ExternalInput")
        w_d[f"ws{l}"] = nc.dram_tensor(
            f"ws{l}", [D + 1, m], fdt, kind="ExternalInput"
        )
    out_d = nc.dram_tensor("out", [PSLAB, 1], dt.float32, kind="ExternalOutput")

    # Split gather tables per layer so section-A gathers (rows [0, 32768),
    # fully inside allgather piece A) depend ONLY on piece A:
    #   hextE: piece-A region [0, BASE_B), written by the early collective.
    #   hextL: the B-section window [BBASE, TOTROW): rows [0, COPY_LEN)
    #          copied locally from hextE, tail written by piece B.
    # Separate tensors per layer so a piece-A write never races the
    # previous layer's in-flight gathers.
    COPY_LEN = BASE_B - BBASE  # 23552
    hextE_ds = [
        nc.dram_tensor(f"hexte{i}", [BASE_B, ROW], fdt, addr_space="Shared")
        for i in range(2)
    ]
    hextL_ds = [
        nc.dram_tensor(f"hextl{i}", [32768, ROW], fdt, addr_space="Shared")
        for i in range(2)
    ]
    slab_d = nc.dram_tensor("slab", [PSLAB2, ROW], fdt)

    bw_env = int(os.environ.get("SAGE_BW", "1"))
    batches = []
    w0 = 0
    while w0 < NW:
        bw = min(bw_env, NW - w0)
        batches.append((w0, bw))
        w0 += bw
    n_layers = int(os.environ.get("SAGE_LAYERS", "3"))
    n_batch_lim = int(os.environ.get("SAGE_BATCHES", str(len(batches))))
    batches = batches[:n_batch_lim]
    no_cc = os.environ.get("SAGE_NOCC", "") == "1"

    with tile.TileContext(nc) as tc:
        with (
            tc.tile_pool(name="const", bufs=1) as cpool,
            tc.tile_pool(
                name="gpool", bufs=int(os.environ.get("SAGE_GBUFS", "8"))
            ) as gpool,
            tc.tile_pool(
                name="spool", bufs=int(os.environ.get("SAGE_SPOOL", "4"))
            ) as spool,
            tc.tile_pool(
                name="psA", bufs=int(os.environ.get("SAGE_PSA", "2")),
                space="PSUM",
            ) as psA,
            tc.tile_pool(name="psB", bufs=2, space="PSUM") as psB,
            tc.tile_pool(name="psC", bufs=2, space="PSUM") as psC,
        ):
            gsems = (
                [nc.alloc_semaphore(f"gsem{q}") for q in range(nqueues)]
                if use_prep else None
            )
            idx_sb = cpool.tile([128, (T_A + T_B) // 16], dt.int16, tag="idx")
            oh_sb = cpool.tile([128, NW * nblk * 128], dt.bfloat16, tag="oh")
            bones_sb = cpool.tile([128, 32], fdt, tag="bones")
            zeros_sb = cpool.tile([128, max(64, max_fill * D)], fdt, tag="zeros")
            zfill_sb = cpool.tile([128, 32], fdt, tag="zfill")
            ident_sb = cpool.tile([WIN, WIN], fdt, tag="ident")
            hs = [cpool.tile([D + 1, PSLAB], fdt, tag=f"hs{i}", name=f"hs{i}")
                  for i in range(3)]
            w_sb = {}
            for l, m in ((0, D), (1, D), (2, 1)):
                w_sb[f"wl{l}"] = cpool.tile([D, m], fdt, tag=f"wl{l}",
                                            name=f"wl{l}")
                w_sb[f"ws{l}"] = cpool.tile([D + 1, m], fdt, tag=f"ws{l}",
                                            name=f"ws{l}")
            zpad_sb = cpool.tile([PADN, ROW], fdt, tag="zpad")

            nc.sync.dma_start(idx_sb[:], idx_d[:])
            nc.sync.dma_start(oh_sb[:], oh_d[:])
            nc.sync.dma_start(bones_sb[:], bones_d[:])
            nc.sync.dma_start(ident_sb[:], ident_d[:])
            nc.sync.dma_start(hs[0][:], xsT_d[:])
            for l in range(3):
                nc.sync.dma_start(w_sb[f"wl{l}"][:], w_d[f"wl{l}"][:])
                nc.sync.dma_start(w_sb[f"ws{l}"][:], w_d[f"ws{l}"][:])
            nc.vector.memset(zpad_sb[:], 0.0)
            nc.vector.memset(zeros_sb[:], 0.0)
            nc.vector.memset(zfill_sb[:], 0.0)
            # bias only on real-node columns: pad columns then compute to
            # exactly 0 (relu(0)), so the slab pad rows need no re-zeroing
            nc.vector.memset(hs[1][D : D + 1, 0:SLAB], 1.0)
            nc.vector.memset(hs[1][D : D + 1, SLAB:PSLAB], 0.0)
            nc.vector.memset(hs[2][D : D + 1, 0:SLAB], 1.0)
            nc.vector.memset(hs[2][D : D + 1, SLAB:PSLAB], 0.0)
            # piece-A pad rows of the slab: zeroed once, never written again
            nc.sync.dma_start(slab_d[H1:H1P, :], zpad_sb[:])

            import contextlib
            reps = int(os.environ.get("SAGE_REPS", "1"))
            psa_bufs = int(os.environ.get("SAGE_PSA", "2"))
            win_counter = 0  # windows emitted; fills only needed while the
            # psA pool's physical tiles are fresh (cells never written later
            # stay zero in PSUM forever)
            rep_cm = (tc.For_i(0, reps, 1, name="reploop")
                      if reps > 1 else contextlib.nullcontext())
            npre = min(int(os.environ.get("SAGE_PRE", "8")),
                       int(os.environ.get("SAGE_GBUFS", "8")), len(batches))
            with rep_cm:
                gtiles = {}
                for layer in range(n_layers):
                    src_a = xext_d[:] if layer == 0 else hextE_ds[layer - 1][:]
                    src_b = (xext_d[BBASE:, :] if layer == 0
                             else hextL_ds[layer - 1][:])
                    hself = hs[layer]
                    wl_t = w_sb[f"wl{layer}"]
                    ws_t = w_sb[f"ws{layer}"]
                    m_out = 1 if layer == 2 else D

                    def emit_gA(bi, src_a=src_a):
                        w0, bw = batches[bi]
                        gA = gpool.tile([128, bw * nch_a, ROW], fdt, tag="gA")
                        gtiles[(bi, "A")] = gA
                        numA = bw * S_A
                        a0 = w0 * S_A // 16
                        nc.gpsimd.dma_gather(
                            gA[:], src_a,
                            idx_sb[:, a0 : a0 + numA // 16],
                            numA, numA, ROW,
                            single_packet=False,
                            queue_num=bi % nqueues,
                        )

                    def emit_gB(bi, src_b=src_b):
                        w0, bw = batches[bi]
                        gB = gpool.tile([128, bw * nch_b, ROW], fdt, tag="gB")
                        gtiles[(bi, "B")] = gB
                        numB = bw * S_B
                        b0c = (T_A + w0 * S_B) // 16
                        nc.gpsimd.dma_gather(
                            gB[:], src_b,
                            idx_sb[:, b0c : b0c + numB // 16],
                            numB, numB, ROW,
                            single_packet=False,
                            queue_num=(bi + 2) % nqueues,
                        )

                    # prefix: A-gathers first (for layer>0 they only need
                    # piece A, which landed mid-previous-layer), THEN the
                    # previous layer's deferred copy + piece-B collective,
                    # then the B-gathers that depend on them.  This keeps
                    # the DMA queues fed across the layer transition.
                    for bi in range(npre):
                        emit_gA(bi)
                    if layer >= 1 and not no_cc:
                        nc.sync.dma_start(
                            hextL_ds[layer - 1][0:COPY_LEN, :],
                            hextE_ds[layer - 1][BBASE:BASE_B, :],
                        )
                        nc.gpsimd.collective_compute(
                            "AllGather",
                            mybir.AluOpType.bypass,
                            replica_groups=[list(range(NCORES))],
                            ins=[slab_d[H1P:PSLAB2]],
                            outs=[hextL_ds[layer - 1][COPY_LEN:32768]],
                        )
                    for bi in range(npre):
                        emit_gB(bi)

                    for bi, (w0, bw) in enumerate(batches):
                        if bi >= npre:
                            emit_gA(bi)
                            emit_gB(bi)
                        gA = gtiles.pop((bi, "A"))
                        gB = gtiles.pop((bi, "B"))

                        stage = int(os.environ.get("SAGE_STAGE", "9"))
                        for wi in range(bw):
                            if stage < 1:
                                break
                            w = w0 + wi
                            gsum_ps = psA.tile([128, nblk * D], dt.float32, tag="gsum")
                            # level 1: 8 wide block-ones matmuls (one per
                            # quadrant x section) over contiguous chunk runs
                            for r in range(4):
                                rr = slice(32 * r, 32 * r + 32)
                                if nA[r]:
                                    a0c = wi * nch_a + int(aoff[r])
                                    nc.tensor.matmul(
                                        gsum_ps[rr, 0 : nA[r] * D],
                                        bones_sb[:],
                                        gA[:, a0c : a0c + nA[r], 0:D],
                                        start=True, stop=True,
                                        tile_position=(0, 32 * r),
                                    )
                                if nB[r]:
                                    b0r = wi * nch_b + int(boff[r])
                                    nc.tensor.matmul(
                                        gsum_ps[rr, nA[r] * D : (nA[r] + nB[r]) * D],
                                        bones_sb[:],
                                        gB[:, b0r : b0r + nB[r], 0:D],
                                        start=True, stop=True,
                                        tile_position=(0, 32 * r),
                                    )
                                fill = nblk - nA[r] - nB[r]
                                if fill and (reps > 1 or win_counter < psa_bufs):
                                    nc.tensor.matmul(
                                        gsum_ps[rr, (nA[r] + nB[r]) * D : nblk * D],
                                        zfill_sb[:],
                                        zeros_sb[:, 0 : fill * D],
                                        start=True, stop=True,
                                        tile_position=(0, 32 * r),
                                    )
                            win_counter += 1
                            if stage < 2:
                                continue
                            # cast to bf16 (1/deg already folded into onehot)
                            gsum_sb = spool.tile([128, nblk * D], dt.bfloat16,
                                                 tag="gsum_sb")
                            if wi % 2 == 0:
                                nc.scalar.activation(
                                    gsum_sb[:], gsum_ps[:],
                                    mybir.ActivationFunctionType.Copy,
                                )
                            else:
                                nc.vector.tensor_copy(gsum_sb[:], gsum_ps[:])
                            if stage < 3:
                                continue
                            # level 2: one-hot accumulate -> meanT [D, 128] scaled
                            win_ps = psB.tile([D, WIN], dt.float32, tag="winps")
                            for blk in range(nblk):
                                oc = (w * nblk + blk) * 128
                                nc.tensor.matmul(
                                    win_ps[:],
                                    gsum_sb[:, blk * D : (blk + 1) * D],
                                    oh_sb[:, oc : oc + 128],
                                    start=(blk == 0), stop=(blk == nblk - 1),
                                )
                            if stage < 4:
                                continue
                            mean_sb = spool.tile([D, WIN], fdt, tag="mean")
                            nc.vector.tensor_copy(mean_sb[:], win_ps[:])
                            # dense, node-major: y = meanT.T@Wl + hselfT.T@Ws_ext
                            y_ps = psC.tile([WIN, m_out], dt.float32, tag="ypsum")
                            nc.tensor.matmul(y_ps[:], mean_sb[:], wl_t[:],
                                             start=True, stop=False)
                            nc.tensor.matmul(y_ps[:],
                                             hself[:, w * WIN : (w + 1) * WIN],
                                             ws_t[:], start=False, stop=True)
                            if layer < 2:
                                hn_sb = spool.tile([WIN, D], fdt, tag="hn")
                                nc.scalar.activation(
                                    hn_sb[:], y_ps[:],
                                    mybir.ActivationFunctionType.Relu,
                                )
                                sr = _srow(w)
                                nc.sync.dma_start(
                                    slab_d[sr : sr + WIN, 0:D], hn_sb[:]
                                )
                                t_ps = psB.tile([D, WIN], fdt, tag="tps",
                                                name="t_ps")
                                nc.tensor.transpose(t_ps[:], hn_sb[:], ident_sb[:])
                                nc.vector.tensor_copy(
                                    hs[layer + 1][0:D, w * WIN : (w + 1) * WIN],
                                    t_ps[:],
                                )
                            else:
                                y_sb = spool.tile([WIN, 1], dt.float32, tag="ysb")
                                nc.scalar.activation(
                                    y_sb[:], y_ps[:],
                                    mybir.ActivationFunctionType.Relu,
                                )
                                nc.sync.dma_start(
                                    out_d[w * WIN : (w + 1) * WIN, :], y_sb[:]
                                )

                        if (layer < 2 and layer < n_layers - 1 and not no_cc
                                and w0 + bw == SPLIT_W):
                            # piece A: windows [0, SPLIT_W) + zero pad rows;
                            # overlaps the remaining windows' gathers/compute.
                            # (the copy + piece B are deferred into the next
                            # layer's prefix, behind its A-gathers)
                            nc.gpsimd.collective_compute(
                                "AllGather",
                                mybir.AluOpType.bypass,
                                replica_groups=[list(range(NCORES))],
                                ins=[slab_d[0:H1P]],
                                outs=[hextE_ds[layer][0 : NCORES * H1P]],
                            )

    nc.compile()
    return nc


def kernel(**inputs):
    x = np.asarray(inputs["x"], dtype=np.float32)
    edge_index = np.asarray(inputs["edge_index"])
    use_bf16 = os.environ.get("SAGE_F32", "") != "1"

    deg = np.bincount(np.asarray(edge_index[1], dtype=np.int64), minlength=N)
    scale = np.where(deg > 0, 1.0 / np.maximum(deg, 1), 0.0).astype(np.float32)

    nch_a, nch_b, nblk, xext, cores = _pack(x, edge_index, scale, use_bf16)

    key = (nch_a, nch_b, nblk, use_bf16)
    if key not in _NC_CACHE:
        _NC_CACHE[key] = _build_nc(nch_a, nch_b, nblk, use_bf16)
    nc = _NC_CACHE[key]

    fdt = ml_dtypes.bfloat16 if use_bf16 else np.float32
    bones = np.kron(np.eye(32), np.ones((4, 1))).astype(fdt)
    ident = np.eye(WIN, dtype=fdt)

    common = {
        "xext": xext,
        "bones": bones,
        "ident": ident,
    }
    for l in range(3):
        common[f"wl{l}"] = np.asarray(inputs[f"Wl{l}"]).astype(fdt)
        wse = np.concatenate(
            [
                np.asarray(inputs[f"Ws{l}"], np.float32),
                (np.asarray(inputs[f"bl{l}"], np.float32)
                 + np.asarray(inputs[f"bs{l}"], np.float32)).reshape(1, -1),
            ],
            axis=0,
        )
        common[f"ws{l}"] = wse.astype(fdt)

    in_maps = []
    for k in range(NCORES):
        m = dict(common)
        m.update(cores[k])
        in_maps.append(m)

    from concourse.bass_utils import run_bass_kernel_spmd

    res = run_bass_kernel_spmd(nc, in_maps, core_ids=list(range(NCORES)))
    global LAST_RESULTS
    LAST_RESULTS = res
    outs = [np.asarray(res.results[k]["out"]).reshape(-1)[:SLAB]
            for k in range(NCORES)]
    return np.concatenate(outs).reshape(N, 1).astype(np.float32)


if __name__ == "__main__":
    pass



# revision 53
# speedup vs baseline: 1.1259x; 1.1259x over previous
"""Trainium2 Bass kernel for 3-layer GraphSAGE (nn_MCHCGraphSage).

Strategy (8 NeuronCores, SPMD single program):
  - Destination-sharded edges: core k owns dst nodes [k*6250, (k+1)*6250).
  - Features live in HBM as 256B rows in "split-slab address" space
    (_addr): each core's slab is stored [windows 0..39 | 22 zero pad rows |
    windows 40..48] so the inter-layer AllGather goes out in two contiguous
    pieces — piece A (rows [0,5142)) fires mid-layer and overlaps the tail
    windows, only piece B (1152 rows) sits on the layer boundary. hext is
    double-buffered (hext0/hext1) so a piece-A write never races the
    previous layer's in-flight gathers.
  - Random x[src] rows are fetched with gpsimd dma_gather (int16 indices)
    spread round-robin over 4 SWDGE queues (the aggregate random-256B
    packet rate ~4 ns/packet is the kernel's bottleneck). int16 range
    forces a two-section split: section A gathers rows [0, 32768),
    section B rows [BBASE, TOTROW) (base offset BBASE).
  - Segmented mean via two PE matmul levels over dst-sorted, degree-padded
    (multiple of 4) edge slots:
      level 1: 8 wide matmuls per window (constant block-ones lhsT
               [128, 32], one per PE row-quadrant x section) over
               run-contiguous chunk slices; never-written PSUM cells are
               zero-filled once (first psA-pool rotation) and stay zero.
      level 2: host-built one-hot [128 groups, 128 dst] (bf16) with the
               1/deg mean scale folded into its values, accumulated in
               PSUM; one PSUM->SBUF bf16 cast per window (ACT/DVE
               alternating).
  - Dense part per window, node-major: y = meanT.T @ Wl + hselfT.T @ Ws_ext
    (bias folded as an extra ones-row of hselfT, zeroed on pad columns so
    pad slab rows compute to exactly 0), ReLU on ACT, DMA the [128, 64]
    node-major block straight to the own slab; PE-transpose to keep the
    feature-major self slab for the next layer.
"""

import os
import sys

import numpy as np

for _p in ("/opt/trn_rl_repo", "/root/.axon_site/_ro/trn_rl_repo"):
    if os.path.isdir(_p) and _p not in sys.path:
        sys.path.append(_p)

import ml_dtypes  # noqa: E402

N = 50000
D = 64
NCORES = 8
SLAB = 6250
PSLAB = 6272
WIN = 128
NW = PSLAB // WIN  # 49
# Split-slab layout: the per-core slab is stored as
#   [windows 0..SPLIT_W-1 (H1 rows) | 22 zero pad rows | windows SPLIT_W..48]
# so the inter-layer AllGather can go out in two contiguous pieces: piece A
# (rows [0, H1P)) fires once windows < SPLIT_W are done and overlaps the
# tail windows' compute; only piece B sits on the layer boundary.
SPLIT_W = 40
H1 = SPLIT_W * WIN  # 5120
PADN = PSLAB - SLAB  # 22
H1P = H1 + PADN  # 5142 (piece-A rows per core, incl. always-zero pad)
H2 = PSLAB - H1  # 1152 (piece-B rows per core)
PSLAB2 = H1P + H2  # 6294 stored slab rows per core
TOTROW = NCORES * PSLAB2  # 50352
BASE_B = NCORES * H1P  # 41136, start of piece-B region in hext
BBASE = TOTROW - 32768  # 17584, B-section base row
APAD_ROW = H1  # row 5120 (core 0 piece-A pad) is always zero
BPAD_ROW = BASE_B + (SLAB - H1)  # core 0's s=6250 pad row, always zero
BW = 4  # windows per gather batch

_NC_CACHE = {}
LAST_RESULTS = None  # test harness introspection (exec_time_ns, profile)


def _addr(n):
    s = n % SLAB
    k = n // SLAB
    return np.where(s < H1, k * H1P + s, BASE_B + k * H2 + (s - H1))


def _srow(w):
    """Stored slab row of window w's first node."""
    return w * WIN if w < SPLIT_W else H1P + (w - SPLIT_W) * WIN


def _run_split(nch_a, nch_b):
    """Assign the NCH chunks of a window to 4 PE row-quadrants in
    contiguous runs: A chunks split [nA0..nA3], then B chunks [nB0..nB3].
    Quadrant r holds A-run r at col-blocks [0, nA[r]) and B-run r at
    [nA[r], nA[r]+nB[r])."""
    nA = np.array([(nch_a + 3 - r) // 4 for r in range(4)])
    nB = np.array([(nch_b + 3 - r) // 4 for r in range(4)])
    aoff = np.concatenate([[0], np.cumsum(nA)]).astype(np.int64)
    boff = np.concatenate([[0], np.cumsum(nB)]).astype(np.int64)
    runmapA = np.repeat(np.arange(4), nA)
    runmapB = np.repeat(np.arange(4), nB)
    nblk = max(a + b for a, b in zip(nA, nB))
    return nA, nB, aoff, boff, runmapA, runmapB, nblk


def _pack(x, edge_index, scale, use_bf16):
    """Host-side packing. Returns per-core dicts + structure constants."""
    src = np.asarray(edge_index[0], dtype=np.int64)
    dst = np.asarray(edge_index[1], dtype=np.int64)
    addr_e = _addr(src)

    # pass 1: global section sizes
    nch_a = 0
    nch_b = 0
    per_core = []
    for k in range(NCORES):
        sel = (dst >= k * SLAB) & (dst < (k + 1) * SLAB)
        s_k = src[sel]
        d_k = dst[sel] - k * SLAB
        a_k = addr_e[sel]
        isA = a_k <= 32767
        degA = np.bincount(d_k[isA], minlength=PSLAB)
        degB = np.bincount(d_k[~isA], minlength=PSLAB)
        padA = ((degA + 3) // 4) * 4
        padB = ((degB + 3) // 4) * 4
        wA = padA.reshape(NW, WIN).sum(1).max()
        wB = padB.reshape(NW, WIN).sum(1).max()
        nch_a = max(nch_a, (int(wA) + 127) // 128)
        nch_b = max(nch_b, (int(wB) + 127) // 128)
        per_core.append((d_k, a_k, isA, padA, padB))

    S_A = nch_a * 128
    S_B = nch_b * 128
    nA, nB, aoff, boff, runmapA, runmapB, NBLK = _run_split(nch_a, nch_b)
    fdt = ml_dtypes.bfloat16 if use_bf16 else np.float32
    ROW = 128 if use_bf16 else 64

    # xext: node features in padded-slab address space, same for all cores
    xext = np.zeros((TOTROW, ROW), dtype=fdt)
    rows = _addr(np.arange(N))
    xext[rows, :D] = x.astype(fdt)

    cores = []
    for k in range(NCORES):
        d_k, a_k, isA, padA, padB = per_core[k]
        pA2 = padA.reshape(NW, WIN)
        pB2 = padB.reshape(NW, WIN)
        offA = (np.cumsum(pA2, 1) - pA2).reshape(-1)  # per local dst
        offB = (np.cumsum(pB2, 1) - pB2).reshape(-1)

        def build_stream(mask, off, S, base, padval):
            e_d = d_k[mask]
            e_a = a_k[mask]
            order = np.argsort(e_d, kind="stable")
            d_s = e_d[order]
            a_s = e_a[order]
            deg = np.bincount(e_d, minlength=PSLAB)
            start = np.concatenate([[0], np.cumsum(deg)])[:-1]
            rank = np.arange(len(d_s)) - start[d_s]
            pos = (d_s // WIN) * S + off[d_s] + rank
            stream = np.full(NW * S, padval, dtype=np.int64)
            stream[pos] = a_s - base
            return stream

        streamA = build_stream(isA, offA, S_A, 0, APAD_ROW)
        streamB = build_stream(~isA, offB, S_B, BBASE, BPAD_ROW - BBASE)
        assert streamA.max() <= 32767 and streamB.max() <= 32767
        assert streamA.min() >= 0 and streamB.min() >= 0

        # group -> (partition, col-block) map with 1/deg folded into the
        # one-hot. Chunks are assigned to PE row-quadrants in contiguous
        # runs (A chunks split [nA0..nA3], B chunks [nB0..nB3]) so level-1
        # can be 8 wide matmuls per window over contiguous chunk slices.
        onehot = np.zeros((128, NW * NBLK * 128), dtype=ml_dtypes.bfloat16)

        def add_section(pad, off, runmap, roff, cb_of):
            reps = pad // 4
            tot = int(reps.sum())
            if tot == 0:
                return
            dstrep = np.repeat(np.arange(PSLAB), reps)
            cum = np.cumsum(reps) - reps
            within = np.arange(tot) - np.repeat(cum, reps)
            gsec = (off // 4)[dstrep] + within  # group idx in window-section
            ca = gsec // 32  # chunk within window-section
            gin = gsec % 32  # group within chunk
            r = runmap[ca]
            p = ca - roff[r]  # position within the quadrant's run
            part = 32 * r + gin
            wnum = dstrep // WIN
            cb = cb_of(r, p)
            cols = (wnum * NBLK + cb) * 128 + dstrep % WIN
            onehot[part, cols] = scale[k * SLAB + dstrep]

        add_section(padA, offA, runmapA, aoff, lambda r, p: p)
        if nch_b > 0:
            add_section(padB, offB, runmapB, boff, lambda r, p: nA[r] + p)

        stream = np.concatenate([streamA, streamB]).astype(np.int16)
        idx16 = stream.reshape(-1, 16).T.copy()  # [16, T/16]
        idx = np.tile(idx16, (8, 1))  # replicate for 8 gpsimd cores

        xselfT = np.zeros((D + 1, PSLAB), dtype=fdt)
        xselfT[:D, :SLAB] = x[k * SLAB : (k + 1) * SLAB].T.astype(fdt)
        xselfT[D, :SLAB] = 1.0  # bias row; pad columns stay 0 -> relu(0)=0

        cores.append({"idx": idx, "onehot": onehot, "xselfT": xselfT})

    return nch_a, nch_b, NBLK, xext, cores


def _build_nc(nch_a, nch_b, nblk, use_bf16):
    import concourse.bacc as bacc
    import concourse.tile as tile
    import concourse.mybir as mybir

    dt = mybir.dt
    fdt = dt.bfloat16 if use_bf16 else dt.float32
    ROW = 128 if use_bf16 else 64
    NCH = nch_a + nch_b
    S_A = nch_a * 128
    S_B = nch_b * 128
    T_A = NW * S_A
    T_B = NW * S_B
    nA, nB, aoff, boff, _, _, nblk_chk = _run_split(nch_a, nch_b)
    assert nblk_chk == nblk
    max_fill = max(nblk - a - b for a, b in zip(nA, nB))

    nqueues = int(os.environ.get("SAGE_QUEUES", "4"))
    use_prep = os.environ.get("SAGE_PREP", "") == "1"
    nc = bacc.Bacc(None, num_devices=NCORES, num_swdge_queues=nqueues)

    xext_d = nc.dram_tensor("xext", [TOTROW, ROW], fdt, kind="ExternalInput")
    idx_d = nc.dram_tensor(
        "idx", [128, (T_A + T_B) // 16], dt.int16, kind="ExternalInput"
    )
    oh_d = nc.dram_tensor(
        "onehot", [128, NW * nblk * 128], dt.bfloat16, kind="ExternalInput"
    )
    xsT_d = nc.dram_tensor("xselfT", [D + 1, PSLAB], fdt, kind="ExternalInput")
    bones_d = nc.dram_tensor("bones", [128, 32], fdt, kind="ExternalInput")
    ident_d = nc.dram_tensor("ident", [WIN, WIN], fdt, kind="ExternalInput")
    w_d = {}
    for l, m in ((0, D), (1, D), (2, 1)):
        w_d[f"wl{l}"] = nc.dram_tensor(f"wl{l}", [D, m], fdt, kind="ExternalInput")
        w_d[f"ws{l}"] = nc.dram_tensor(
            f"ws{l}", [D + 1, m], fdt, kind="ExternalInput"
        )
    out_d = nc.dram_tensor("out", [PSLAB, 1], dt.float32, kind="ExternalOutput")

    # double-buffered so layer L+1's allgather (piece A, fired mid-layer)
    # never overwrites the table layer L's late gathers are still reading
    hext_ds = [
        nc.dram_tensor(f"hext{i}", [TOTROW, ROW], fdt, addr_space="Shared")
        for i in range(2)
    ]
    slab_d = nc.dram_tensor("slab", [PSLAB2, ROW], fdt)

    bw_env = int(os.environ.get("SAGE_BW", "1"))
    batches = []
    w0 = 0
    while w0 < NW:
        bw = min(bw_env, NW - w0)
        batches.append((w0, bw))
        w0 += bw
    n_layers = int(os.environ.get("SAGE_LAYERS", "3"))
    n_batch_lim = int(os.environ.get("SAGE_BATCHES", str(len(batches))))
    batches = batches[:n_batch_lim]
    no_cc = os.environ.get("SAGE_NOCC", "") == "1"

    with tile.TileContext(nc) as tc:
        with (
            tc.tile_pool(name="const", bufs=1) as cpool,
            tc.tile_pool(
                name="gpool", bufs=int(os.environ.get("SAGE_GBUFS", "8"))
            ) as gpool,
            tc.tile_pool(
                name="spool", bufs=int(os.environ.get("SAGE_SPOOL", "4"))
            ) as spool,
            tc.tile_pool(
                name="psA", bufs=int(os.environ.get("SAGE_PSA", "2")),
                space="PSUM",
            ) as psA,
            tc.tile_pool(name="psB", bufs=2, space="PSUM") as psB,
            tc.tile_pool(name="psC", bufs=2, space="PSUM") as psC,
        ):
            gsems = (
                [nc.alloc_semaphore(f"gsem{q}") for q in range(nqueues)]
                if use_prep else None
            )
            idx_sb = cpool.tile([128, (T_A + T_B) // 16], dt.int16, tag="idx")
            oh_sb = cpool.tile([128, NW * nblk * 128], dt.bfloat16, tag="oh")
            bones_sb = cpool.tile([128, 32], fdt, tag="bones")
            zeros_sb = cpool.tile([128, max(64, max_fill * D)], fdt, tag="zeros")
            zfill_sb = cpool.tile([128, 32], fdt, tag="zfill")
            ident_sb = cpool.tile([WIN, WIN], fdt, tag="ident")
            hs = [cpool.tile([D + 1, PSLAB], fdt, tag=f"hs{i}", name=f"hs{i}")
                  for i in range(3)]
            w_sb = {}
            for l, m in ((0, D), (1, D), (2, 1)):
                w_sb[f"wl{l}"] = cpool.tile([D, m], fdt, tag=f"wl{l}",
                                            name=f"wl{l}")
                w_sb[f"ws{l}"] = cpool.tile([D + 1, m], fdt, tag=f"ws{l}",
                                            name=f"ws{l}")
            zpad_sb = cpool.tile([PADN, ROW], fdt, tag="zpad")

            nc.sync.dma_start(idx_sb[:], idx_d[:])
            nc.sync.dma_start(oh_sb[:], oh_d[:])
            nc.sync.dma_start(bones_sb[:], bones_d[:])
            nc.sync.dma_start(ident_sb[:], ident_d[:])
            nc.sync.dma_start(hs[0][:], xsT_d[:])
            for l in range(3):
                nc.sync.dma_start(w_sb[f"wl{l}"][:], w_d[f"wl{l}"][:])
                nc.sync.dma_start(w_sb[f"ws{l}"][:], w_d[f"ws{l}"][:])
            nc.vector.memset(zpad_sb[:], 0.0)
            nc.vector.memset(zeros_sb[:], 0.0)
            nc.vector.memset(zfill_sb[:], 0.0)
            # bias only on real-node columns: pad columns then compute to
            # exactly 0 (relu(0)), so the slab pad rows need no re-zeroing
            nc.vector.memset(hs[1][D : D + 1, 0:SLAB], 1.0)
            nc.vector.memset(hs[1][D : D + 1, SLAB:PSLAB], 0.0)
            nc.vector.memset(hs[2][D : D + 1, 0:SLAB], 1.0)
            nc.vector.memset(hs[2][D : D + 1, SLAB:PSLAB], 0.0)
            # piece-A pad rows of the slab: zeroed once, never written again
            nc.sync.dma_start(slab_d[H1:H1P, :], zpad_sb[:])

            import contextlib
            reps = int(os.environ.get("SAGE_REPS", "1"))
            psa_bufs = int(os.environ.get("SAGE_PSA", "2"))
            win_counter = 0  # windows emitted; fills only needed while the
            # psA pool's physical tiles are fresh (cells never written later
            # stay zero in PSUM forever)
            rep_cm = (tc.For_i(0, reps, 1, name="reploop")
                      if reps > 1 else contextlib.nullcontext())
            with rep_cm:
                for layer in range(n_layers):
                    src_t = xext_d if layer == 0 else hext_ds[layer - 1]
                    hself = hs[layer]
                    wl_t = w_sb[f"wl{layer}"]
                    ws_t = w_sb[f"ws{layer}"]
                    m_out = 1 if layer == 2 else D

                    for bi, (w0, bw) in enumerate(batches):
                        gA = gpool.tile([128, bw * nch_a, ROW], fdt, tag="gA")
                        gB = gpool.tile([128, bw * nch_b, ROW], fdt, tag="gB")
                        numA = bw * S_A
                        numB = bw * S_B
                        a0 = w0 * S_A // 16
                        b0c = (T_A + w0 * S_B) // 16
                        qA = (2 * bi) % nqueues
                        qB = (2 * bi + 1) % nqueues
                        if use_prep:
                            nc.gpsimd.dma_gather(
                                gA[:], src_t[:],
                                idx_sb[:, a0 : a0 + numA // 16],
                                numA, numA, ROW,
                                single_packet=False,
                                queue_num=qA,
                                prepare_only=True, sem=gsems[qA],
                            )
                            nc.gpsimd.trigger_dma(count=None, queue_num=qA)
                            nc.gpsimd.dma_gather(
                                gB[:], src_t[BBASE:, :],
                                idx_sb[:, b0c : b0c + numB // 16],
                                numB, numB, ROW,
                                single_packet=False,
                                queue_num=qB,
                                prepare_only=True, sem=gsems[qB],
                            )
                            nc.gpsimd.trigger_dma(count=None, queue_num=qB)
                        else:
                            nc.gpsimd.dma_gather(
                                gA[:], src_t[:],
                                idx_sb[:, a0 : a0 + numA // 16],
                                numA, numA, ROW,
                                single_packet=False,
                                queue_num=qA,
                            )
                            nc.gpsimd.dma_gather(
                                gB[:], src_t[BBASE:, :],
                                idx_sb[:, b0c : b0c + numB // 16],
                                numB, numB, ROW,
                                single_packet=False,
                                queue_num=qB,
                            )

                        stage = int(os.environ.get("SAGE_STAGE", "9"))
                        for wi in range(bw):
                            if stage < 1:
                                break
                            w = w0 + wi
                            gsum_ps = psA.tile([128, nblk * D], dt.float32, tag="gsum")
                            # level 1: 8 wide block-ones matmuls (one per
                            # quadrant x section) over contiguous chunk runs
                            for r in range(4):
                                rr = slice(32 * r, 32 * r + 32)
                                if nA[r]:
                                    a0c = wi * nch_a + int(aoff[r])
                                    nc.tensor.matmul(
                                        gsum_ps[rr, 0 : nA[r] * D],
                                        bones_sb[:],
                                        gA[:, a0c : a0c + nA[r], 0:D],
                                        start=True, stop=True,
                                        tile_position=(0, 32 * r),
                                    )
                                if nB[r]:
                                    b0r = wi * nch_b + int(boff[r])
                                    nc.tensor.matmul(
                                        gsum_ps[rr, nA[r] * D : (nA[r] + nB[r]) * D],
                                        bones_sb[:],
                                        gB[:, b0r : b0r + nB[r], 0:D],
                                        start=True, stop=True,
                                        tile_position=(0, 32 * r),
                                    )
                                fill = nblk - nA[r] - nB[r]
                                if fill and (reps > 1 or win_counter < psa_bufs):
                                    nc.tensor.matmul(
                                        gsum_ps[rr, (nA[r] + nB[r]) * D : nblk * D],
                                        zfill_sb[:],
                                        zeros_sb[:, 0 : fill * D],
                                        start=True, stop=True,
                                        tile_position=(0, 32 * r),
                                    )
                            win_counter += 1
                            if stage < 2:
                                continue
                            # cast to bf16 (1/deg already folded into onehot)
                            gsum_sb = spool.tile([128, nblk * D], dt.bfloat16,
                                                 tag="gsum_sb")
                            if wi % 2 == 0:
                                nc.scalar.activation(
                                    gsum_sb[:], gsum_ps[:],
                                    mybir.ActivationFunctionType.Copy,
                                )
                            else:
                                nc.vector.tensor_copy(gsum_sb[:], gsum_ps[:])
                            if stage < 3:
                                continue
                            # level 2: one-hot accumulate -> meanT [D, 128] scaled
                            win_ps = psB.tile([D, WIN], dt.float32, tag="winps")
                            for blk in range(nblk):
                                oc = (w * nblk + blk) * 128
                                nc.tensor.matmul(
                                    win_ps[:],
                                    gsum_sb[:, blk * D : (blk + 1) * D],
                                    oh_sb[:, oc : oc + 128],
                                    start=(blk == 0), stop=(blk == nblk - 1),
                                )
                            if stage < 4:
                                continue
                            mean_sb = spool.tile([D, WIN], fdt, tag="mean")
                            nc.vector.tensor_copy(mean_sb[:], win_ps[:])
                            # dense, node-major: y = meanT.T@Wl + hselfT.T@Ws_ext
                            y_ps = psC.tile([WIN, m_out], dt.float32, tag="ypsum")
                            nc.tensor.matmul(y_ps[:], mean_sb[:], wl_t[:],
                                             start=True, stop=False)
                            nc.tensor.matmul(y_ps[:],
                                             hself[:, w * WIN : (w + 1) * WIN],
                                             ws_t[:], start=False, stop=True)
                            if layer < 2:
                                hn_sb = spool.tile([WIN, D], fdt, tag="hn")
                                nc.scalar.activation(
                                    hn_sb[:], y_ps[:],
                                    mybir.ActivationFunctionType.Relu,
                                )
                                sr = _srow(w)
                                nc.sync.dma_start(
                                    slab_d[sr : sr + WIN, 0:D], hn_sb[:]
                                )
                                t_ps = psB.tile([D, WIN], fdt, tag="tps",
                                                name="t_ps")
                                nc.tensor.transpose(t_ps[:], hn_sb[:], ident_sb[:])
                                nc.vector.tensor_copy(
                                    hs[layer + 1][0:D, w * WIN : (w + 1) * WIN],
                                    t_ps[:],
                                )
                            else:
                                y_sb = spool.tile([WIN, 1], dt.float32, tag="ysb")
                                nc.scalar.activation(
                                    y_sb[:], y_ps[:],
                                    mybir.ActivationFunctionType.Relu,
                                )
                                nc.sync.dma_start(
                                    out_d[w * WIN : (w + 1) * WIN, :], y_sb[:]
                                )

                        if (layer < 2 and layer < n_layers - 1 and not no_cc
                                and w0 + bw == SPLIT_W):
                            # piece A: windows [0, SPLIT_W) + zero pad rows;
                            # overlaps the remaining windows' gathers/compute
                            nc.gpsimd.collective_compute(
                                "AllGather",
                                mybir.AluOpType.bypass,
                                replica_groups=[list(range(NCORES))],
                                ins=[slab_d[0:H1P]],
                                outs=[hext_ds[layer][0 : NCORES * H1P]],
                            )

                    if layer < 2 and layer < n_layers - 1 and not no_cc:
                        nc.gpsimd.collective_compute(
                            "AllGather",
                            mybir.AluOpType.bypass,
                            replica_groups=[list(range(NCORES))],
                            ins=[slab_d[H1P:PSLAB2]],
                            outs=[hext_ds[layer][BASE_B:TOTROW]],
                        )

    nc.compile()
    return nc


def kernel(**inputs):
    x = np.asarray(inputs["x"], dtype=np.float32)
    edge_index = np.asarray(inputs["edge_index"])
    use_bf16 = os.environ.get("SAGE_F32", "") != "1"

    deg = np.bincount(np.asarray(edge_index[1], dtype=np.int64), minlength=N)
    scale = np.where(deg > 0, 1.0 / np.maximum(deg, 1), 0.0).astype(np.float32)

    nch_a, nch_b, nblk, xext, cores = _pack(x, edge_index, scale, use_bf16)

    key = (nch_a, nch_b, nblk, use_bf16)
    if key not in _NC_CACHE:
        _NC_CACHE[key] = _build_nc(nch_a, nch_b, nblk, use_bf16)
    nc = _NC_CACHE[key]

    fdt = ml_dtypes.bfloat16 if use_bf16 else np.float32
    bones = np.kron(np.eye(32), np.ones((4, 1))).astype(fdt)
    ident = np.eye(WIN, dtype=fdt)

    common = {
        "xext": xext,
        "bones": bones,
        "ident": ident,
    }
    for l in range(3):
        common[f"wl{l}"] = np.asarray(inputs[f"Wl{l}"]).astype(fdt)
        wse = np.concatenate(
            [
                np.asarray(inputs[f"Ws{l}"], np.float32),
                (np.asarray(inputs[f"bl{l}"], np.float32)
                 + np.asarray(inputs[f"bs{l}"], np.float32)).reshape(1, -1),
            ],
            axis=0,
        )
        common[f"ws{l}"] = wse.astype(fdt)

    in_maps = []
    for k in range(NCORES):
        m = dict(common)
        m.update(cores[k])
        in_maps.append(m)

    from concourse.bass_utils import run_bass_kernel_spmd

    res = run_bass_kernel_spmd(nc, in_maps, core_ids=list(range(NCORES)))
    global LAST_RESULTS
    LAST_RESULTS = res
    outs = [np.asarray(res.results[k]["out"]).reshape(-1)[:SLAB]
            for k in range(NCORES)]
    return np.concatenate(outs).reshape(N, 1).astype(np.float32)


if __name__ == "__main__":
    pass



# revision 56
# speedup vs baseline: 1.1774x; 1.0458x over previous
"""Trainium2 Bass kernel for 3-layer GraphSAGE (nn_MCHCGraphSage).

Strategy (8 NeuronCores, SPMD single program):
  - Destination-sharded edges: core k owns dst nodes [k*6250, (k+1)*6250).
  - Features live in HBM as 256B rows in "split-slab address" space
    (_addr): each core's slab is stored [windows 0..39 | 22 zero pad rows |
    windows 40..48] so the inter-layer AllGather goes out in two contiguous
    pieces — piece A (rows [0,5142)) fires mid-layer and overlaps the tail
    windows, only piece B (1152 rows) sits on the layer boundary. hext is
    double-buffered (hext0/hext1) so a piece-A write never races the
    previous layer's in-flight gathers.
  - Random x[src] rows are fetched with gpsimd dma_gather (int16 indices)
    spread round-robin over 4 SWDGE queues (the aggregate random-256B
    packet rate ~4 ns/packet is the kernel's bottleneck). int16 range
    forces a two-section split: section A gathers rows [0, 32768),
    section B rows [BBASE, TOTROW) (base offset BBASE).
  - Segmented mean via two PE matmul levels over dst-sorted, degree-padded
    (multiple of 4) edge slots:
      level 1: 8 wide matmuls per window (constant block-ones lhsT
               [128, 32], one per PE row-quadrant x section) over
               run-contiguous chunk slices; never-written PSUM cells are
               zero-filled once (first psA-pool rotation) and stay zero.
      level 2: host-built one-hot [128 groups, 128 dst] (bf16) with the
               1/deg mean scale folded into its values, accumulated in
               PSUM; one PSUM->SBUF bf16 cast per window (ACT/DVE
               alternating).
  - Dense part per window, node-major: y = meanT.T @ Wl + hselfT.T @ Ws_ext
    (bias folded as an extra ones-row of hselfT, zeroed on pad columns so
    pad slab rows compute to exactly 0), ReLU on ACT, DMA the [128, 64]
    node-major block straight to the own slab; PE-transpose to keep the
    feature-major self slab for the next layer.
"""

import os
import sys

import numpy as np

for _p in ("/opt/trn_rl_repo", "/root/.axon_site/_ro/trn_rl_repo"):
    if os.path.isdir(_p) and _p not in sys.path:
        sys.path.append(_p)

import ml_dtypes  # noqa: E402

N = 50000
D = 64
NCORES = 8
SLAB = 6250
PSLAB = 6272
WIN = 128
NW = PSLAB // WIN  # 49
# Split-slab layout: the per-core slab is stored as
#   [windows 0..SPLIT_W-1 (H1 rows) | 22 zero pad rows | windows SPLIT_W..48]
# so the inter-layer AllGather can go out in two contiguous pieces: piece A
# (rows [0, H1P)) fires once windows < SPLIT_W are done and overlaps the
# tail windows' compute; only piece B sits on the layer boundary.
SPLIT_W = 40
H1 = SPLIT_W * WIN  # 5120
PADN = PSLAB - SLAB  # 22
H1P = H1 + PADN  # 5142 (piece-A rows per core, incl. always-zero pad)
H2 = PSLAB - H1  # 1152 (piece-B rows per core)
PSLAB2 = H1P + H2  # 6294 stored slab rows per core
TOTROW = NCORES * PSLAB2  # 50352
BASE_B = NCORES * H1P  # 41136, start of piece-B region in hext
BBASE = TOTROW - 32768  # 17584, B-section base row
APAD_ROW = H1  # row 5120 (core 0 piece-A pad) is always zero
BPAD_ROW = BASE_B + (SLAB - H1)  # core 0's s=6250 pad row, always zero
BW = 4  # windows per gather batch

_NC_CACHE = {}
LAST_RESULTS = None  # test harness introspection (exec_time_ns, profile)


def _addr(n):
    s = n % SLAB
    k = n // SLAB
    return np.where(s < H1, k * H1P + s, BASE_B + k * H2 + (s - H1))


def _srow(w):
    """Stored slab row of window w's first node."""
    return w * WIN if w < SPLIT_W else H1P + (w - SPLIT_W) * WIN


def _run_split(nch_a, nch_b):
    """Assign the NCH chunks of a window to 4 PE row-quadrants in
    contiguous runs: A chunks split [nA0..nA3], then B chunks [nB0..nB3].
    Quadrant r holds A-run r at col-blocks [0, nA[r]) and B-run r at
    [nA[r], nA[r]+nB[r])."""
    # A runs descending, B runs ascending across quadrants so the
    # per-quadrant totals (and thus nblk) stay minimal
    nA = np.array([(nch_a + 3 - r) // 4 for r in range(4)])
    nB = np.array([(nch_b + r) // 4 for r in range(4)])
    aoff = np.concatenate([[0], np.cumsum(nA)]).astype(np.int64)
    boff = np.concatenate([[0], np.cumsum(nB)]).astype(np.int64)
    runmapA = np.repeat(np.arange(4), nA)
    runmapB = np.repeat(np.arange(4), nB)
    nblk = max(a + b for a, b in zip(nA, nB))
    return nA, nB, aoff, boff, runmapA, runmapB, nblk


def _balance_sections(d_k, a_k):
    """Per-dst section assignment minimizing pad-to-4 waste.

    Addresses in [BBASE, 32768) are reachable by BOTH int16 sections.  For
    each dst, route its overlap edges to whichever section brings both
    per-(dst, section) degrees to a multiple of 4 — this removes most of
    the two-section padding (~1.5 pad slots per dst per section).
    """
    fixedA = a_k < BBASE
    fixedB = a_k > 32767
    flex = (~fixedA) & (~fixedB)
    nfA = np.bincount(d_k[fixedA], minlength=PSLAB)
    nfB = np.bincount(d_k[fixedB], minlength=PSLAB)
    nflex = np.bincount(d_k[flex], minlength=PSLAB)

    # candidates cover all kA residues near the 50/50-balancing target
    # (equal section sizes minimize the two per-window max-chunk ceils),
    # plus the dA==0 / dB==0 extremes
    ntot = nfA + nfB + nflex
    kTgt = np.clip((ntot + 1) // 2 - nfA, 0, nflex)
    cands = [np.zeros_like(nflex), nflex.copy()]
    for t in range(-3, 4):
        cands.append(np.clip(kTgt + t, 0, nflex))
    best_key = None
    best_k = np.zeros(PSLAB, dtype=np.int64)
    for kA in cands:
        dA = nfA + kA
        dB = nfB + nflex - kA
        cost = np.where(dA > 0, (-dA) % 4, 0) + np.where(dB > 0, (-dB) % 4, 0)
        key = cost * 4096 + np.abs(kA - kTgt)
        if best_key is None:
            best_key, best_k = key, kA
        else:
            upd = key < best_key
            best_k = np.where(upd, kA, best_k)
            best_key = np.minimum(key, best_key)

    fi = np.flatnonzero(flex)
    order = np.argsort(d_k[fi], kind="stable")
    fid = fi[order]
    dd = d_k[fid]
    start = np.concatenate(
        [[0], np.cumsum(np.bincount(dd, minlength=PSLAB))]
    )[:-1]
    rank = np.arange(len(fid)) - start[dd]
    isA = fixedA.copy()
    isA[fid[rank < best_k[dd]]] = True
    return isA


def _pack(x, edge_index, scale, use_bf16):
    """Host-side packing. Returns per-core dicts + structure constants."""
    src = np.asarray(edge_index[0], dtype=np.int64)
    dst = np.asarray(edge_index[1], dtype=np.int64)
    addr_e = _addr(src)

    # pass 1: global section sizes
    nch_a = 0
    nch_b = 0
    per_core = []
    for k in range(NCORES):
        sel = (dst >= k * SLAB) & (dst < (k + 1) * SLAB)
        s_k = src[sel]
        d_k = dst[sel] - k * SLAB
        a_k = addr_e[sel]
        isA = _balance_sections(d_k, a_k)
        degA = np.bincount(d_k[isA], minlength=PSLAB)
        degB = np.bincount(d_k[~isA], minlength=PSLAB)
        padA = ((degA + 3) // 4) * 4
        padB = ((degB + 3) // 4) * 4
        wA = padA.reshape(NW, WIN).sum(1).max()
        wB = padB.reshape(NW, WIN).sum(1).max()
        nch_a = max(nch_a, (int(wA) + 127) // 128)
        nch_b = max(nch_b, (int(wB) + 127) // 128)
        per_core.append((d_k, a_k, isA, padA, padB))

    S_A = nch_a * 128
    S_B = nch_b * 128
    nA, nB, aoff, boff, runmapA, runmapB, NBLK = _run_split(nch_a, nch_b)
    fdt = ml_dtypes.bfloat16 if use_bf16 else np.float32
    ROW = 128 if use_bf16 else 64

    # xext: node features in padded-slab address space, same for all cores
    xext = np.zeros((TOTROW, ROW), dtype=fdt)
    rows = _addr(np.arange(N))
    xext[rows, :D] = x.astype(fdt)

    cores = []
    for k in range(NCORES):
        d_k, a_k, isA, padA, padB = per_core[k]
        pA2 = padA.reshape(NW, WIN)
        pB2 = padB.reshape(NW, WIN)
        offA = (np.cumsum(pA2, 1) - pA2).reshape(-1)  # per local dst
        offB = (np.cumsum(pB2, 1) - pB2).reshape(-1)

        def build_stream(mask, off, S, base, padval):
            e_d = d_k[mask]
            e_a = a_k[mask]
            order = np.argsort(e_d, kind="stable")
            d_s = e_d[order]
            a_s = e_a[order]
            deg = np.bincount(e_d, minlength=PSLAB)
            start = np.concatenate([[0], np.cumsum(deg)])[:-1]
            rank = np.arange(len(d_s)) - start[d_s]
            pos = (d_s // WIN) * S + off[d_s] + rank
            stream = np.full(NW * S, padval, dtype=np.int64)
            stream[pos] = a_s - base
            return stream

        streamA = build_stream(isA, offA, S_A, 0, APAD_ROW)
        streamB = build_stream(~isA, offB, S_B, BBASE, BPAD_ROW - BBASE)
        assert streamA.max() <= 32767 and streamB.max() <= 32767
        assert streamA.min() >= 0 and streamB.min() >= 0

        # group -> (partition, col-block) map with 1/deg folded into the
        # one-hot. Chunks are assigned to PE row-quadrants in contiguous
        # runs (A chunks split [nA0..nA3], B chunks [nB0..nB3]) so level-1
        # can be 8 wide matmuls per window over contiguous chunk slices.
        onehot = np.zeros((128, NW * NBLK * 128), dtype=ml_dtypes.bfloat16)

        def add_section(pad, off, runmap, roff, cb_of):
            reps = pad // 4
            tot = int(reps.sum())
            if tot == 0:
                return
            dstrep = np.repeat(np.arange(PSLAB), reps)
            cum = np.cumsum(reps) - reps
            within = np.arange(tot) - np.repeat(cum, reps)
            gsec = (off // 4)[dstrep] + within  # group idx in window-section
            ca = gsec // 32  # chunk within window-section
            gin = gsec % 32  # group within chunk
            r = runmap[ca]
            p = ca - roff[r]  # position within the quadrant's run
            part = 32 * r + gin
            wnum = dstrep // WIN
            cb = cb_of(r, p)
            cols = (wnum * NBLK + cb) * 128 + dstrep % WIN
            onehot[part, cols] = scale[k * SLAB + dstrep]

        add_section(padA, offA, runmapA, aoff, lambda r, p: p)
        if nch_b > 0:
            add_section(padB, offB, runmapB, boff, lambda r, p: nA[r] + p)

        stream = np.concatenate([streamA, streamB]).astype(np.int16)
        idx16 = stream.reshape(-1, 16).T.copy()  # [16, T/16]
        idx = np.tile(idx16, (8, 1))  # replicate for 8 gpsimd cores

        xselfT = np.zeros((D + 1, PSLAB), dtype=fdt)
        xselfT[:D, :SLAB] = x[k * SLAB : (k + 1) * SLAB].T.astype(fdt)
        xselfT[D, :SLAB] = 1.0  # bias row; pad columns stay 0 -> relu(0)=0

        cores.append({"idx": idx, "onehot": onehot, "xselfT": xselfT})

    return nch_a, nch_b, NBLK, xext, cores


def _build_nc(nch_a, nch_b, nblk, use_bf16):
    import concourse.bacc as bacc
    import concourse.tile as tile
    import concourse.mybir as mybir

    dt = mybir.dt
    fdt = dt.bfloat16 if use_bf16 else dt.float32
    ROW = 128 if use_bf16 else 64
    NCH = nch_a + nch_b
    S_A = nch_a * 128
    S_B = nch_b * 128
    T_A = NW * S_A
    T_B = NW * S_B
    nA, nB, aoff, boff, _, _, nblk_chk = _run_split(nch_a, nch_b)
    assert nblk_chk == nblk
    max_fill = max(nblk - a - b for a, b in zip(nA, nB))

    nqueues = int(os.environ.get("SAGE_QUEUES", "4"))
    use_prep = os.environ.get("SAGE_PREP", "") == "1"
    nc = bacc.Bacc(None, num_devices=NCORES, num_swdge_queues=nqueues)

    xext_d = nc.dram_tensor("xext", [TOTROW, ROW], fdt, kind="ExternalInput")
    idx_d = nc.dram_tensor(
        "idx", [128, (T_A + T_B) // 16], dt.int16, kind="ExternalInput"
    )
    oh_d = nc.dram_tensor(
        "onehot", [128, NW * nblk * 128], dt.bfloat16, kind="ExternalInput"
    )
    xsT_d = nc.dram_tensor("xselfT", [D + 1, PSLAB], fdt, kind="ExternalInput")
    bones_d = nc.dram_tensor("bones", [128, 32], fdt, kind="ExternalInput")
    ident_d = nc.dram_tensor("ident", [WIN, WIN], fdt, kind="ExternalInput")
    w_d = {}
    for l, m in ((0, D), (1, D), (2, 1)):
        w_d[f"wl{l}"] = nc.dram_tensor(f"wl{l}", [D, m], fdt, kind="ExternalInput")
        w_d[f"ws{l}"] = nc.dram_tensor(
            f"ws{l}", [D + 1, m], fdt, kind="ExternalInput"
        )
    out_d = nc.dram_tensor("out", [PSLAB, 1], dt.float32, kind="ExternalOutput")

    # double-buffered so layer L+1's allgather (piece A, fired mid-layer)
    # never overwrites the table layer L's late gathers are still reading
    hext_ds = [
        nc.dram_tensor(f"hext{i}", [TOTROW, ROW], fdt, addr_space="Shared")
        for i in range(2)
    ]
    slab_d = nc.dram_tensor("slab", [PSLAB2, ROW], fdt)

    bw_env = int(os.environ.get("SAGE_BW", "1"))
    batches = []
    w0 = 0
    while w0 < NW:
        bw = min(bw_env, NW - w0)
        batches.append((w0, bw))
        w0 += bw
    n_layers = int(os.environ.get("SAGE_LAYERS", "3"))
    n_batch_lim = int(os.environ.get("SAGE_BATCHES", str(len(batches))))
    batches = batches[:n_batch_lim]
    no_cc = os.environ.get("SAGE_NOCC", "") == "1"

    with tile.TileContext(nc) as tc:
        with (
            tc.tile_pool(name="const", bufs=1) as cpool,
            tc.tile_pool(
                name="gpool", bufs=int(os.environ.get("SAGE_GBUFS", "8"))
            ) as gpool,
            tc.tile_pool(
                name="spool", bufs=int(os.environ.get("SAGE_SPOOL", "4"))
            ) as spool,
            tc.tile_pool(
                name="psA", bufs=int(os.environ.get("SAGE_PSA", "2")),
                space="PSUM",
            ) as psA,
            tc.tile_pool(name="psB", bufs=2, space="PSUM") as psB,
            tc.tile_pool(name="psC", bufs=2, space="PSUM") as psC,
        ):
            gsems = (
                [nc.alloc_semaphore(f"gsem{q}") for q in range(nqueues)]
                if use_prep else None
            )
            idx_sb = cpool.tile([128, (T_A + T_B) // 16], dt.int16, tag="idx")
            oh_sb = cpool.tile([128, NW * nblk * 128], dt.bfloat16, tag="oh")
            bones_sb = cpool.tile([128, 32], fdt, tag="bones")
            zeros_sb = cpool.tile([128, max(64, max_fill * D)], fdt, tag="zeros")
            zfill_sb = cpool.tile([128, 32], fdt, tag="zfill")
            ident_sb = cpool.tile([WIN, WIN], fdt, tag="ident")
            hs = [cpool.tile([D + 1, PSLAB], fdt, tag=f"hs{i}", name=f"hs{i}")
                  for i in range(3)]
            w_sb = {}
            for l, m in ((0, D), (1, D), (2, 1)):
                w_sb[f"wl{l}"] = cpool.tile([D, m], fdt, tag=f"wl{l}",
                                            name=f"wl{l}")
                w_sb[f"ws{l}"] = cpool.tile([D + 1, m], fdt, tag=f"ws{l}",
                                            name=f"ws{l}")
            zpad_sb = cpool.tile([PADN, ROW], fdt, tag="zpad")

            nc.sync.dma_start(idx_sb[:], idx_d[:])
            nc.sync.dma_start(oh_sb[:], oh_d[:])
            nc.sync.dma_start(bones_sb[:], bones_d[:])
            nc.sync.dma_start(ident_sb[:], ident_d[:])
            nc.sync.dma_start(hs[0][:], xsT_d[:])
            for l in range(3):
                nc.sync.dma_start(w_sb[f"wl{l}"][:], w_d[f"wl{l}"][:])
                nc.sync.dma_start(w_sb[f"ws{l}"][:], w_d[f"ws{l}"][:])
            nc.vector.memset(zpad_sb[:], 0.0)
            nc.vector.memset(zeros_sb[:], 0.0)
            nc.vector.memset(zfill_sb[:], 0.0)
            # bias only on real-node columns: pad columns then compute to
            # exactly 0 (relu(0)), so the slab pad rows need no re-zeroing
            nc.vector.memset(hs[1][D : D + 1, 0:SLAB], 1.0)
            nc.vector.memset(hs[1][D : D + 1, SLAB:PSLAB], 0.0)
            nc.vector.memset(hs[2][D : D + 1, 0:SLAB], 1.0)
            nc.vector.memset(hs[2][D : D + 1, SLAB:PSLAB], 0.0)
            # piece-A pad rows of the slab: zeroed once, never written again
            nc.sync.dma_start(slab_d[H1:H1P, :], zpad_sb[:])

            import contextlib
            reps = int(os.environ.get("SAGE_REPS", "1"))
            psa_bufs = int(os.environ.get("SAGE_PSA", "2"))
            win_counter = 0  # windows emitted; fills only needed while the
            # psA pool's physical tiles are fresh (cells never written later
            # stay zero in PSUM forever)
            rep_cm = (tc.For_i(0, reps, 1, name="reploop")
                      if reps > 1 else contextlib.nullcontext())
            with rep_cm:
                for layer in range(n_layers):
                    src_t = xext_d if layer == 0 else hext_ds[layer - 1]
                    hself = hs[layer]
                    wl_t = w_sb[f"wl{layer}"]
                    ws_t = w_sb[f"ws{layer}"]
                    m_out = 1 if layer == 2 else D

                    for bi, (w0, bw) in enumerate(batches):
                        gA = gpool.tile([128, bw * nch_a, ROW], fdt, tag="gA")
                        gB = gpool.tile([128, bw * nch_b, ROW], fdt, tag="gB")
                        numA = bw * S_A
                        numB = bw * S_B
                        a0 = w0 * S_A // 16
                        b0c = (T_A + w0 * S_B) // 16
                        qA = (2 * bi) % nqueues
                        qB = (2 * bi + 1) % nqueues
                        if use_prep:
                            nc.gpsimd.dma_gather(
                                gA[:], src_t[:],
                                idx_sb[:, a0 : a0 + numA // 16],
                                numA, numA, ROW,
                                single_packet=False,
                                queue_num=qA,
                                prepare_only=True, sem=gsems[qA],
                            )
                            nc.gpsimd.trigger_dma(count=None, queue_num=qA)
                            nc.gpsimd.dma_gather(
                                gB[:], src_t[BBASE:, :],
                                idx_sb[:, b0c : b0c + numB // 16],
                                numB, numB, ROW,
                                single_packet=False,
                                queue_num=qB,
                                prepare_only=True, sem=gsems[qB],
                            )
                            nc.gpsimd.trigger_dma(count=None, queue_num=qB)
                        else:
                            nc.gpsimd.dma_gather(
                                gA[:], src_t[:],
                                idx_sb[:, a0 : a0 + numA // 16],
                                numA, numA, ROW,
                                single_packet=False,
                                queue_num=qA,
                            )
                            nc.gpsimd.dma_gather(
                                gB[:], src_t[BBASE:, :],
                                idx_sb[:, b0c : b0c + numB // 16],
                                numB, numB, ROW,
                                single_packet=False,
                                queue_num=qB,
                            )

                        stage = int(os.environ.get("SAGE_STAGE", "9"))
                        for wi in range(bw):
                            if stage < 1:
                                break
                            w = w0 + wi
                            gsum_ps = psA.tile([128, nblk * D], dt.float32, tag="gsum")
                            # level 1: 8 wide block-ones matmuls (one per
                            # quadrant x section) over contiguous chunk runs
                            for r in range(4):
                                rr = slice(32 * r, 32 * r + 32)
                                if nA[r]:
                                    a0c = wi * nch_a + int(aoff[r])
                                    nc.tensor.matmul(
                                        gsum_ps[rr, 0 : nA[r] * D],
                                        bones_sb[:],
                                        gA[:, a0c : a0c + nA[r], 0:D],
                                        start=True, stop=True,
                                        tile_position=(0, 32 * r),
                                    )
                                if nB[r]:
                                    b0r = wi * nch_b + int(boff[r])
                                    nc.tensor.matmul(
                                        gsum_ps[rr, nA[r] * D : (nA[r] + nB[r]) * D],
                                        bones_sb[:],
                                        gB[:, b0r : b0r + nB[r], 0:D],
                                        start=True, stop=True,
                                        tile_position=(0, 32 * r),
                                    )
                                fill = nblk - nA[r] - nB[r]
                                if fill and (reps > 1 or win_counter < psa_bufs):
                                    nc.tensor.matmul(
                                        gsum_ps[rr, (nA[r] + nB[r]) * D : nblk * D],
                                        zfill_sb[:],
                                        zeros_sb[:, 0 : fill * D],
                                        start=True, stop=True,
                                        tile_position=(0, 32 * r),
                                    )
                            win_counter += 1
                            if stage < 2:
                                continue
                            # cast to bf16 (1/deg already folded into onehot)
                            gsum_sb = spool.tile([128, nblk * D], dt.bfloat16,
                                                 tag="gsum_sb")
                            if wi % 2 == 0:
                                nc.scalar.activation(
                                    gsum_sb[:], gsum_ps[:],
                                    mybir.ActivationFunctionType.Copy,
                                )
                            else:
                                nc.vector.tensor_copy(gsum_sb[:], gsum_ps[:])
                            if stage < 3:
                                continue
                            # level 2: one-hot accumulate -> meanT [D, 128] scaled
                            win_ps = psB.tile([D, WIN], dt.float32, tag="winps")
                            for blk in range(nblk):
                                oc = (w * nblk + blk) * 128
                                nc.tensor.matmul(
                                    win_ps[:],
                                    gsum_sb[:, blk * D : (blk + 1) * D],
                                    oh_sb[:, oc : oc + 128],
                                    start=(blk == 0), stop=(blk == nblk - 1),
                                )
                            if stage < 4:
                                continue
                            mean_sb = spool.tile([D, WIN], fdt, tag="mean")
                            nc.vector.tensor_copy(mean_sb[:], win_ps[:])
                            # dense, node-major: y = meanT.T@Wl + hselfT.T@Ws_ext
                            y_ps = psC.tile([WIN, m_out], dt.float32, tag="ypsum")
                            nc.tensor.matmul(y_ps[:], mean_sb[:], wl_t[:],
                                             start=True, stop=False)
                            nc.tensor.matmul(y_ps[:],
                                             hself[:, w * WIN : (w + 1) * WIN],
                                             ws_t[:], start=False, stop=True)
                            if layer < 2:
                                hn_sb = spool.tile([WIN, D], fdt, tag="hn")
                                nc.scalar.activation(
                                    hn_sb[:], y_ps[:],
                                    mybir.ActivationFunctionType.Relu,
                                )
                                sr = _srow(w)
                                nc.sync.dma_start(
                                    slab_d[sr : sr + WIN, 0:D], hn_sb[:]
                                )
                                t_ps = psB.tile([D, WIN], fdt, tag="tps",
                                                name="t_ps")
                                nc.tensor.transpose(t_ps[:], hn_sb[:], ident_sb[:])
                                nc.vector.tensor_copy(
                                    hs[layer + 1][0:D, w * WIN : (w + 1) * WIN],
                                    t_ps[:],
                                )
                            else:
                                y_sb = spool.tile([WIN, 1], dt.float32, tag="ysb")
                                nc.scalar.activation(
                                    y_sb[:], y_ps[:],
                                    mybir.ActivationFunctionType.Relu,
                                )
                                nc.sync.dma_start(
                                    out_d[w * WIN : (w + 1) * WIN, :], y_sb[:]
                                )

                        if (layer < 2 and layer < n_layers - 1 and not no_cc
                                and w0 + bw == SPLIT_W):
                            # piece A: windows [0, SPLIT_W) + zero pad rows;
                            # overlaps the remaining windows' gathers/compute
                            nc.gpsimd.collective_compute(
                                "AllGather",
                                mybir.AluOpType.bypass,
                                replica_groups=[list(range(NCORES))],
                                ins=[slab_d[0:H1P]],
                                outs=[hext_ds[layer][0 : NCORES * H1P]],
                            )

                    if layer < 2 and layer < n_layers - 1 and not no_cc:
                        nc.gpsimd.collective_compute(
                            "AllGather",
                            mybir.AluOpType.bypass,
                            replica_groups=[list(range(NCORES))],
                            ins=[slab_d[H1P:PSLAB2]],
                            outs=[hext_ds[layer][BASE_B:TOTROW]],
                        )

    nc.compile()
    return nc


def kernel(**inputs):
    x = np.asarray(inputs["x"], dtype=np.float32)
    edge_index = np.asarray(inputs["edge_index"])
    use_bf16 = os.environ.get("SAGE_F32", "") != "1"

    deg = np.bincount(np.asarray(edge_index[1], dtype=np.int64), minlength=N)
    scale = np.where(deg > 0, 1.0 / np.maximum(deg, 1), 0.0).astype(np.float32)

    nch_a, nch_b, nblk, xext, cores = _pack(x, edge_index, scale, use_bf16)

    key = (nch_a, nch_b, nblk, use_bf16)
    if key not in _NC_CACHE:
        _NC_CACHE[key] = _build_nc(nch_a, nch_b, nblk, use_bf16)
    nc = _NC_CACHE[key]

    fdt = ml_dtypes.bfloat16 if use_bf16 else np.float32
    bones = np.kron(np.eye(32), np.ones((4, 1))).astype(fdt)
    ident = np.eye(WIN, dtype=fdt)

    common = {
        "xext": xext,
        "bones": bones,
        "ident": ident,
    }
    for l in range(3):
        common[f"wl{l}"] = np.asarray(inputs[f"Wl{l}"]).astype(fdt)
        wse = np.concatenate(
            [
                np.asarray(inputs[f"Ws{l}"], np.float32),
                (np.asarray(inputs[f"bl{l}"], np.float32)
                 + np.asarray(inputs[f"bs{l}"], np.float32)).reshape(1, -1),
            ],
            axis=0,
        )
        common[f"ws{l}"] = wse.astype(fdt)

    in_maps = []
    for k in range(NCORES):
        m = dict(common)
        m.update(cores[k])
        in_maps.append(m)

    from concourse.bass_utils import run_bass_kernel_spmd

    res = run_bass_kernel_spmd(nc, in_maps, core_ids=list(range(NCORES)))
    global LAST_RESULTS
    LAST_RESULTS = res
    outs = [np.asarray(res.results[k]["out"]).reshape(-1)[:SLAB]
            for k in range(NCORES)]
    return np.concatenate(outs).reshape(N, 1).astype(np.float32)


if __name__ == "__main__":
    pass



# revision 60
# speedup vs baseline: 1.3676x; 1.1615x over previous
"""Trainium2 Bass kernel for 3-layer GraphSAGE (nn_MCHCGraphSage).

Strategy (8 NeuronCores, SPMD single program):
  - Destination-sharded edges: core k owns dst nodes [k*6250, (k+1)*6250).
  - Features live in HBM as 256B rows in "split-slab address" space
    (_addr): each core's slab is stored [windows 0..39 | 22 zero pad rows |
    windows 40..48] so the inter-layer AllGather goes out in two contiguous
    pieces — piece A (rows [0,5142)) fires mid-layer and overlaps the tail
    windows, only piece B (1152 rows) sits on the layer boundary. hext is
    double-buffered (hext0/hext1) so a piece-A write never races the
    previous layer's in-flight gathers.
  - Random x[src] rows are fetched with gpsimd dma_gather (int16 indices)
    spread round-robin over 4 SWDGE queues (the aggregate random-256B
    packet rate ~4 ns/packet is the kernel's bottleneck). int16 range
    forces a two-section split: section A gathers rows [0, 32768),
    section B rows [BBASE, TOTROW) (base offset BBASE).
  - Segmented mean via two PE matmul levels over dst-sorted, degree-padded
    (multiple of 4) edge slots:
      level 1: 8 wide matmuls per window (constant block-ones lhsT
               [128, 32], one per PE row-quadrant x section) over
               run-contiguous chunk slices; never-written PSUM cells are
               zero-filled once (first psA-pool rotation) and stay zero.
      level 2: host-built one-hot [128 groups, 128 dst] (bf16) with the
               1/deg mean scale folded into its values, accumulated in
               PSUM; one PSUM->SBUF bf16 cast per window (ACT/DVE
               alternating).
  - Dense part per window, node-major: y = meanT.T @ Wl + hselfT.T @ Ws_ext
    (bias folded as an extra ones-row of hselfT, zeroed on pad columns so
    pad slab rows compute to exactly 0), ReLU on ACT, DMA the [128, 64]
    node-major block straight to the own slab; PE-transpose to keep the
    feature-major self slab for the next layer.
"""

import os
import sys

import numpy as np

for _p in ("/opt/trn_rl_repo", "/root/.axon_site/_ro/trn_rl_repo"):
    if os.path.isdir(_p) and _p not in sys.path:
        sys.path.append(_p)

import ml_dtypes  # noqa: E402

N = 50000
D = 64
NCORES = 8
SLAB = 6250
PSLAB = 6272
WIN = 128
NW = PSLAB // WIN  # 49
# Split-slab layout: the per-core slab is stored as
#   [windows 0..SPLIT_W-1 (H1 rows) | 22 zero pad rows | windows SPLIT_W..48]
# so the inter-layer AllGather can go out in two contiguous pieces: piece A
# (rows [0, H1P)) fires once windows < SPLIT_W are done and overlaps the
# tail windows' compute; only piece B sits on the layer boundary.
SPLIT_W = 40
H1 = SPLIT_W * WIN  # 5120
PADN = PSLAB - SLAB  # 22
H1P = H1 + PADN  # 5142 (piece-A rows per core, incl. always-zero pad)
H2 = PSLAB - H1  # 1152 (piece-B rows per core)
PSLAB2 = H1P + H2  # 6294 stored slab rows per core
TOTROW = NCORES * PSLAB2  # 50352
BASE_B = NCORES * H1P  # 41136, start of piece-B region in hext
BBASE = TOTROW - 32768  # 17584, B-section base row
APAD_ROW = H1  # row 5120 (core 0 piece-A pad) is always zero
BPAD_ROW = BASE_B + (SLAB - H1)  # core 0's s=6250 pad row, always zero
BW = 4  # windows per gather batch

_NC_CACHE = {}
LAST_RESULTS = None  # test harness introspection (exec_time_ns, profile)


def _addr(n):
    s = n % SLAB
    k = n // SLAB
    return np.where(s < H1, k * H1P + s, BASE_B + k * H2 + (s - H1))


def _srow(w):
    """Stored slab row of window w's first node."""
    return w * WIN if w < SPLIT_W else H1P + (w - SPLIT_W) * WIN


def _run_split(nch_a, nch_b):
    """Assign the NCH chunks of a window to 4 PE row-quadrants in
    contiguous runs: A chunks split [nA0..nA3], then B chunks [nB0..nB3].
    Quadrant r holds A-run r at col-blocks [0, nA[r]) and B-run r at
    [nA[r], nA[r]+nB[r])."""
    # A runs descending, B runs ascending across quadrants so the
    # per-quadrant totals (and thus nblk) stay minimal
    nA = np.array([(nch_a + 3 - r) // 4 for r in range(4)])
    nB = np.array([(nch_b + r) // 4 for r in range(4)])
    aoff = np.concatenate([[0], np.cumsum(nA)]).astype(np.int64)
    boff = np.concatenate([[0], np.cumsum(nB)]).astype(np.int64)
    runmapA = np.repeat(np.arange(4), nA)
    runmapB = np.repeat(np.arange(4), nB)
    nblk = max(a + b for a, b in zip(nA, nB))
    return nA, nB, aoff, boff, runmapA, runmapB, nblk


def _balance_sections(d_k, a_k):
    """Per-dst section assignment minimizing pad-to-4 waste.

    Addresses in [BBASE, 32768) are reachable by BOTH int16 sections.  For
    each dst, route its overlap edges to whichever section brings both
    per-(dst, section) degrees to a multiple of 4 — this removes most of
    the two-section padding (~1.5 pad slots per dst per section).
    """
    fixedA = a_k < BBASE
    fixedB = a_k > 32767
    flex = (~fixedA) & (~fixedB)
    nfA = np.bincount(d_k[fixedA], minlength=PSLAB)
    nfB = np.bincount(d_k[fixedB], minlength=PSLAB)
    nflex = np.bincount(d_k[flex], minlength=PSLAB)

    # candidates cover all kA residues near the 50/50-balancing target
    # (equal section sizes minimize the two per-window max-chunk ceils),
    # plus the dA==0 / dB==0 extremes
    ntot = nfA + nfB + nflex
    kTgt = np.clip((ntot + 1) // 2 - nfA, 0, nflex)
    cands = [np.zeros_like(nflex), nflex.copy()]
    for t in range(-3, 4):
        cands.append(np.clip(kTgt + t, 0, nflex))
    best_key = None
    best_k = np.zeros(PSLAB, dtype=np.int64)
    for kA in cands:
        dA = nfA + kA
        dB = nfB + nflex - kA
        cost = np.where(dA > 0, (-dA) % 4, 0) + np.where(dB > 0, (-dB) % 4, 0)
        key = cost * 4096 + np.abs(kA - kTgt)
        if best_key is None:
            best_key, best_k = key, kA
        else:
            upd = key < best_key
            best_k = np.where(upd, kA, best_k)
            best_key = np.minimum(key, best_key)

    fi = np.flatnonzero(flex)
    order = np.argsort(d_k[fi], kind="stable")
    fid = fi[order]
    dd = d_k[fid]
    start = np.concatenate(
        [[0], np.cumsum(np.bincount(dd, minlength=PSLAB))]
    )[:-1]
    rank = np.arange(len(fid)) - start[dd]
    isA = fixedA.copy()
    isA[fid[rank < best_k[dd]]] = True
    return isA


def _pack(x, edge_index, scale, use_bf16):
    """Host-side packing. Returns per-core dicts + structure constants."""
    src = np.asarray(edge_index[0], dtype=np.int64)
    dst = np.asarray(edge_index[1], dtype=np.int64)
    addr_e = _addr(src)

    # pass 1: global section sizes
    nch_a = 0
    nch_b = 0
    per_core = []
    for k in range(NCORES):
        sel = (dst >= k * SLAB) & (dst < (k + 1) * SLAB)
        s_k = src[sel]
        d_k = dst[sel] - k * SLAB
        a_k = addr_e[sel]
        isA = _balance_sections(d_k, a_k)
        degA = np.bincount(d_k[isA], minlength=PSLAB)
        degB = np.bincount(d_k[~isA], minlength=PSLAB)
        padA = ((degA + 3) // 4) * 4
        padB = ((degB + 3) // 4) * 4
        wA = padA.reshape(NW, WIN).sum(1).max()
        wB = padB.reshape(NW, WIN).sum(1).max()
        nch_a = max(nch_a, (int(wA) + 127) // 128)
        nch_b = max(nch_b, (int(wB) + 127) // 128)
        per_core.append((d_k, a_k, isA, padA, padB))

    S_A = nch_a * 128
    S_B = nch_b * 128
    nA, nB, aoff, boff, runmapA, runmapB, NBLK = _run_split(nch_a, nch_b)
    fdt = ml_dtypes.bfloat16 if use_bf16 else np.float32
    ROW = 128 if use_bf16 else 64

    # xext: node features in padded-slab address space, same for all cores
    xext = np.zeros((TOTROW, ROW), dtype=fdt)
    rows = _addr(np.arange(N))
    xext[rows, :D] = x.astype(fdt)

    cores = []
    for k in range(NCORES):
        d_k, a_k, isA, padA, padB = per_core[k]
        pA2 = padA.reshape(NW, WIN)
        pB2 = padB.reshape(NW, WIN)
        offA = (np.cumsum(pA2, 1) - pA2).reshape(-1)  # per local dst
        offB = (np.cumsum(pB2, 1) - pB2).reshape(-1)

        def build_stream(mask, off, S, base, padval):
            e_d = d_k[mask]
            e_a = a_k[mask]
            order = np.argsort(e_d, kind="stable")
            d_s = e_d[order]
            a_s = e_a[order]
            deg = np.bincount(e_d, minlength=PSLAB)
            start = np.concatenate([[0], np.cumsum(deg)])[:-1]
            rank = np.arange(len(d_s)) - start[d_s]
            pos = (d_s // WIN) * S + off[d_s] + rank
            stream = np.full(NW * S, padval, dtype=np.int64)
            stream[pos] = a_s - base
            return stream

        streamA = build_stream(isA, offA, S_A, 0, APAD_ROW)
        streamB = build_stream(~isA, offB, S_B, BBASE, BPAD_ROW - BBASE)
        assert streamA.max() <= 32767 and streamB.max() <= 32767
        assert streamA.min() >= 0 and streamB.min() >= 0

        # group -> (partition, col-block) map with 1/deg folded into the
        # one-hot. Chunks are assigned to PE row-quadrants in contiguous
        # runs (A chunks split [nA0..nA3], B chunks [nB0..nB3]) so level-1
        # can be 8 wide matmuls per window over contiguous chunk slices.
        onehot = np.zeros((128, NW * NBLK * 128), dtype=ml_dtypes.bfloat16)

        def add_section(pad, off, runmap, roff, cb_of):
            reps = pad // 4
            tot = int(reps.sum())
            if tot == 0:
                return
            dstrep = np.repeat(np.arange(PSLAB), reps)
            cum = np.cumsum(reps) - reps
            within = np.arange(tot) - np.repeat(cum, reps)
            gsec = (off // 4)[dstrep] + within  # group idx in window-section
            ca = gsec // 32  # chunk within window-section
            gin = gsec % 32  # group within chunk
            r = runmap[ca]
            p = ca - roff[r]  # position within the quadrant's run
            part = 32 * r + gin
            wnum = dstrep // WIN
            cb = cb_of(r, p)
            cols = (wnum * NBLK + cb) * 128 + dstrep % WIN
            onehot[part, cols] = scale[k * SLAB + dstrep]

        add_section(padA, offA, runmapA, aoff, lambda r, p: p)
        if nch_b > 0:
            add_section(padB, offB, runmapB, boff, lambda r, p: nA[r] + p)

        stream = np.concatenate([streamA, streamB]).astype(np.int16)
        idx16 = stream.reshape(-1, 16).T.copy()  # [16, T/16]
        idx = np.tile(idx16, (8, 1))  # replicate for 8 gpsimd cores

        xselfT = np.zeros((D + 1, PSLAB), dtype=fdt)
        xselfT[:D, :SLAB] = x[k * SLAB : (k + 1) * SLAB].T.astype(fdt)
        xselfT[D, :SLAB] = 1.0  # bias row; pad columns stay 0 -> relu(0)=0

        cores.append({"idx": idx, "onehot": onehot, "xselfT": xselfT})

    return nch_a, nch_b, NBLK, xext, cores


def _build_nc(nch_a, nch_b, nblk, use_bf16):
    import concourse.bacc as bacc
    import concourse.tile as tile
    import concourse.mybir as mybir

    dt = mybir.dt
    fdt = dt.bfloat16 if use_bf16 else dt.float32
    ROW = 128 if use_bf16 else 64
    NCH = nch_a + nch_b
    S_A = nch_a * 128
    S_B = nch_b * 128
    T_A = NW * S_A
    T_B = NW * S_B
    nA, nB, aoff, boff, _, _, nblk_chk = _run_split(nch_a, nch_b)
    assert nblk_chk == nblk
    max_fill = max(nblk - a - b for a, b in zip(nA, nB))

    nqueues = int(os.environ.get("SAGE_QUEUES", "4"))
    use_prep = os.environ.get("SAGE_PREP", "") == "1"
    nc = bacc.Bacc(None, num_devices=NCORES, num_swdge_queues=nqueues)

    xext_d = nc.dram_tensor("xext", [TOTROW, ROW], fdt, kind="ExternalInput")
    idx_d = nc.dram_tensor(
        "idx", [128, (T_A + T_B) // 16], dt.int16, kind="ExternalInput"
    )
    oh_d = nc.dram_tensor(
        "onehot", [128, NW * nblk * 128], dt.bfloat16, kind="ExternalInput"
    )
    xsT_d = nc.dram_tensor("xselfT", [D + 1, PSLAB], fdt, kind="ExternalInput")
    bones_d = nc.dram_tensor("bones", [128, 32], fdt, kind="ExternalInput")
    ident_d = nc.dram_tensor("ident", [WIN, WIN], fdt, kind="ExternalInput")
    w_d = {}
    for l, m in ((0, D), (1, D), (2, 1)):
        w_d[f"wl{l}"] = nc.dram_tensor(f"wl{l}", [D, m], fdt, kind="ExternalInput")
        w_d[f"ws{l}"] = nc.dram_tensor(
            f"ws{l}", [D + 1, m], fdt, kind="ExternalInput"
        )
    out_d = nc.dram_tensor("out", [PSLAB, 1], dt.float32, kind="ExternalOutput")

    # double-buffered so layer L+1's allgather (piece A, fired mid-layer)
    # never overwrites the table layer L's late gathers are still reading
    hext_ds = [
        nc.dram_tensor(f"hext{i}", [TOTROW, ROW], fdt, addr_space="Shared")
        for i in range(2)
    ]
    slab_d = nc.dram_tensor("slab", [PSLAB2, ROW], fdt)

    bw_env = int(os.environ.get("SAGE_BW", "1"))
    batches = []
    w0 = 0
    while w0 < NW:
        bw = min(bw_env, NW - w0)
        batches.append((w0, bw))
        w0 += bw
    n_layers = int(os.environ.get("SAGE_LAYERS", "3"))
    n_batch_lim = int(os.environ.get("SAGE_BATCHES", str(len(batches))))
    batches = batches[:n_batch_lim]
    no_cc = os.environ.get("SAGE_NOCC", "") == "1"

    with tile.TileContext(nc) as tc:
        with (
            tc.tile_pool(name="const", bufs=1) as cpool,
            tc.tile_pool(
                name="gpool", bufs=int(os.environ.get("SAGE_GBUFS", "8"))
            ) as gpool,
            tc.tile_pool(
                name="spool", bufs=int(os.environ.get("SAGE_SPOOL", "4"))
            ) as spool,
            tc.tile_pool(
                name="psA", bufs=int(os.environ.get("SAGE_PSA", "2")),
                space="PSUM",
            ) as psA,
            tc.tile_pool(name="psB", bufs=2, space="PSUM") as psB,
            tc.tile_pool(name="psC", bufs=2, space="PSUM") as psC,
        ):
            gsems = (
                [nc.alloc_semaphore(f"gsem{q}") for q in range(nqueues)]
                if use_prep else None
            )
            OH1W = 8  # windows covered by the first one-hot tile
            OH1 = OH1W * nblk * 128
            idx_sb = cpool.tile([128, (T_A + T_B) // 16], dt.int16, tag="idx")
            oh_sb1 = cpool.tile([128, OH1], dt.bfloat16, tag="oh1")
            oh_sb2 = cpool.tile(
                [128, NW * nblk * 128 - OH1], dt.bfloat16, tag="oh2"
            )
            bones_sb = cpool.tile([128, 32], fdt, tag="bones")
            zeros_sb = cpool.tile([128, max(64, max_fill * D)], fdt, tag="zeros")
            zfill_sb = cpool.tile([128, 32], fdt, tag="zfill")
            ident_sb = cpool.tile([WIN, WIN], fdt, tag="ident")
            hs = [cpool.tile([D + 1, PSLAB], fdt, tag=f"hs{i}", name=f"hs{i}")
                  for i in range(3)]
            w_sb = {}
            for l, m in ((0, D), (1, D), (2, 1)):
                w_sb[f"wl{l}"] = cpool.tile([D, m], fdt, tag=f"wl{l}",
                                            name=f"wl{l}")
                w_sb[f"ws{l}"] = cpool.tile([D + 1, m], fdt, tag=f"ws{l}",
                                            name=f"ws{l}")
            zpad_sb = cpool.tile([PADN, ROW], fdt, tag="zpad")

            # load order matters for the startup ramp: idx first (gathers
            # need it), then the small dense-path constants, then the big
            # one-hot as two tiles so window 0's level-2 only waits for
            # the slice covering the first OH1W windows
            nc.sync.dma_start(idx_sb[:], idx_d[:])
            nc.sync.dma_start(bones_sb[:], bones_d[:])
            nc.sync.dma_start(ident_sb[:], ident_d[:])
            nc.sync.dma_start(hs[0][:], xsT_d[:])
            for l in range(3):
                nc.sync.dma_start(w_sb[f"wl{l}"][:], w_d[f"wl{l}"][:])
                nc.sync.dma_start(w_sb[f"ws{l}"][:], w_d[f"ws{l}"][:])
            nc.sync.dma_start(oh_sb1[:], oh_d[:, 0:OH1])
            nc.sync.dma_start(oh_sb2[:], oh_d[:, OH1:])
            nc.vector.memset(zpad_sb[:], 0.0)
            nc.vector.memset(zeros_sb[:], 0.0)
            nc.vector.memset(zfill_sb[:], 0.0)
            # bias only on real-node columns: pad columns then compute to
            # exactly 0 (relu(0)), so the slab pad rows need no re-zeroing
            nc.vector.memset(hs[1][D : D + 1, 0:SLAB], 1.0)
            nc.vector.memset(hs[1][D : D + 1, SLAB:PSLAB], 0.0)
            nc.vector.memset(hs[2][D : D + 1, 0:SLAB], 1.0)
            nc.vector.memset(hs[2][D : D + 1, SLAB:PSLAB], 0.0)
            # piece-A pad rows of the slab: zeroed once, never written again
            nc.sync.dma_start(slab_d[H1:H1P, :], zpad_sb[:])

            import contextlib
            reps = int(os.environ.get("SAGE_REPS", "1"))
            psa_bufs = int(os.environ.get("SAGE_PSA", "2"))
            win_counter = 0  # windows emitted; fills only needed while the
            # psA pool's physical tiles are fresh (cells never written later
            # stay zero in PSUM forever)
            rep_cm = (tc.For_i(0, reps, 1, name="reploop")
                      if reps > 1 else contextlib.nullcontext())
            with rep_cm:
                for layer in range(n_layers):
                    src_t = xext_d if layer == 0 else hext_ds[layer - 1]
                    hself = hs[layer]
                    wl_t = w_sb[f"wl{layer}"]
                    ws_t = w_sb[f"ws{layer}"]
                    m_out = 1 if layer == 2 else D

                    for bi, (w0, bw) in enumerate(batches):
                        gA = gpool.tile([128, bw * nch_a, ROW], fdt, tag="gA")
                        gB = gpool.tile([128, bw * nch_b, ROW], fdt, tag="gB")
                        numA = bw * S_A
                        numB = bw * S_B
                        a0 = w0 * S_A // 16
                        b0c = (T_A + w0 * S_B) // 16
                        qA = (2 * bi) % nqueues
                        qB = (2 * bi + 1) % nqueues
                        if use_prep:
                            nc.gpsimd.dma_gather(
                                gA[:], src_t[:],
                                idx_sb[:, a0 : a0 + numA // 16],
                                numA, numA, ROW,
                                single_packet=False,
                                queue_num=qA,
                                prepare_only=True, sem=gsems[qA],
                            )
                            nc.gpsimd.trigger_dma(count=None, queue_num=qA)
                            nc.gpsimd.dma_gather(
                                gB[:], src_t[BBASE:, :],
                                idx_sb[:, b0c : b0c + numB // 16],
                                numB, numB, ROW,
                                single_packet=False,
                                queue_num=qB,
                                prepare_only=True, sem=gsems[qB],
                            )
                            nc.gpsimd.trigger_dma(count=None, queue_num=qB)
                        else:
                            nc.gpsimd.dma_gather(
                                gA[:], src_t[:],
                                idx_sb[:, a0 : a0 + numA // 16],
                                numA, numA, ROW,
                                single_packet=False,
                                queue_num=qA,
                            )
                            nc.gpsimd.dma_gather(
                                gB[:], src_t[BBASE:, :],
                                idx_sb[:, b0c : b0c + numB // 16],
                                numB, numB, ROW,
                                single_packet=False,
                                queue_num=qB,
                            )

                        stage = int(os.environ.get("SAGE_STAGE", "9"))
                        for wi in range(bw):
                            if stage < 1:
                                break
                            w = w0 + wi
                            gsum_ps = psA.tile([128, nblk * D], dt.float32, tag="gsum")
                            # level 1: 8 wide block-ones matmuls (one per
                            # quadrant x section) over contiguous chunk runs
                            for r in range(4):
                                rr = slice(32 * r, 32 * r + 32)
                                if nA[r]:
                                    a0c = wi * nch_a + int(aoff[r])
                                    nc.tensor.matmul(
                                        gsum_ps[rr, 0 : nA[r] * D],
                                        bones_sb[:],
                                        gA[:, a0c : a0c + nA[r], 0:D],
                                        start=True, stop=True,
                                        tile_position=(0, 32 * r),
                                    )
                                if nB[r]:
                                    b0r = wi * nch_b + int(boff[r])
                                    nc.tensor.matmul(
                                        gsum_ps[rr, nA[r] * D : (nA[r] + nB[r]) * D],
                                        bones_sb[:],
                                        gB[:, b0r : b0r + nB[r], 0:D],
                                        start=True, stop=True,
                                        tile_position=(0, 32 * r),
                                    )
                                fill = nblk - nA[r] - nB[r]
                                if fill and (reps > 1 or win_counter < psa_bufs):
                                    nc.tensor.matmul(
                                        gsum_ps[rr, (nA[r] + nB[r]) * D : nblk * D],
                                        zfill_sb[:],
                                        zeros_sb[:, 0 : fill * D],
                                        start=True, stop=True,
                                        tile_position=(0, 32 * r),
                                    )
                            win_counter += 1
                            if stage < 2:
                                continue
                            # cast to bf16 (1/deg already folded into onehot)
                            gsum_sb = spool.tile([128, nblk * D], dt.bfloat16,
                                                 tag="gsum_sb")
                            if wi % 2 == 0:
                                nc.scalar.activation(
                                    gsum_sb[:], gsum_ps[:],
                                    mybir.ActivationFunctionType.Copy,
                                )
                            else:
                                nc.vector.tensor_copy(gsum_sb[:], gsum_ps[:])
                            if stage < 3:
                                continue
                            # level 2: one-hot accumulate -> meanT [D, 128] scaled
                            win_ps = psB.tile([D, WIN], dt.float32, tag="winps")
                            oh_t = oh_sb1 if w < OH1W else oh_sb2
                            ob = 0 if w < OH1W else OH1
                            for blk in range(nblk):
                                oc = (w * nblk + blk) * 128 - ob
                                nc.tensor.matmul(
                                    win_ps[:],
                                    gsum_sb[:, blk * D : (blk + 1) * D],
                                    oh_t[:, oc : oc + 128],
                                    start=(blk == 0), stop=(blk == nblk - 1),
                                )
                            if stage < 4:
                                continue
                            mean_sb = spool.tile([D, WIN], fdt, tag="mean")
                            nc.vector.tensor_copy(mean_sb[:], win_ps[:])
                            # dense, node-major: y = meanT.T@Wl + hselfT.T@Ws_ext
                            y_ps = psC.tile([WIN, m_out], dt.float32, tag="ypsum")
                            nc.tensor.matmul(y_ps[:], mean_sb[:], wl_t[:],
                                             start=True, stop=False)
                            nc.tensor.matmul(y_ps[:],
                                             hself[:, w * WIN : (w + 1) * WIN],
                                             ws_t[:], start=False, stop=True)
                            if layer < 2:
                                hn_sb = spool.tile([WIN, D], fdt, tag="hn")
                                nc.scalar.activation(
                                    hn_sb[:], y_ps[:],
                                    mybir.ActivationFunctionType.Relu,
                                )
                                sr = _srow(w)
                                nc.sync.dma_start(
                                    slab_d[sr : sr + WIN, 0:D], hn_sb[:]
                                )
                                t_ps = psB.tile([D, WIN], fdt, tag="tps",
                                                name="t_ps")
                                nc.tensor.transpose(t_ps[:], hn_sb[:], ident_sb[:])
                                nc.vector.tensor_copy(
                                    hs[layer + 1][0:D, w * WIN : (w + 1) * WIN],
                                    t_ps[:],
                                )
                            else:
                                y_sb = spool.tile([WIN, 1], dt.float32, tag="ysb")
                                nc.scalar.activation(
                                    y_sb[:], y_ps[:],
                                    mybir.ActivationFunctionType.Relu,
                                )
                                nc.sync.dma_start(
                                    out_d[w * WIN : (w + 1) * WIN, :], y_sb[:]
                                )

                        if (layer < 2 and layer < n_layers - 1 and not no_cc
                                and w0 + bw == SPLIT_W):
                            # piece A: windows [0, SPLIT_W) + zero pad rows;
                            # overlaps the remaining windows' gathers/compute
                            nc.gpsimd.collective_compute(
                                "AllGather",
                                mybir.AluOpType.bypass,
                                replica_groups=[list(range(NCORES))],
                                ins=[slab_d[0:H1P]],
                                outs=[hext_ds[layer][0 : NCORES * H1P]],
                            )

                    if layer < 2 and layer < n_layers - 1 and not no_cc:
                        nc.gpsimd.collective_compute(
                            "AllGather",
                            mybir.AluOpType.bypass,
                            replica_groups=[list(range(NCORES))],
                            ins=[slab_d[H1P:PSLAB2]],
                            outs=[hext_ds[layer][BASE_B:TOTROW]],
                        )

    nc.compile()
    return nc


def kernel(**inputs):
    x = np.asarray(inputs["x"], dtype=np.float32)
    edge_index = np.asarray(inputs["edge_index"])
    use_bf16 = os.environ.get("SAGE_F32", "") != "1"

    deg = np.bincount(np.asarray(edge_index[1], dtype=np.int64), minlength=N)
    scale = np.where(deg > 0, 1.0 / np.maximum(deg, 1), 0.0).astype(np.float32)

    nch_a, nch_b, nblk, xext, cores = _pack(x, edge_index, scale, use_bf16)

    key = (nch_a, nch_b, nblk, use_bf16)
    if key not in _NC_CACHE:
        _NC_CACHE[key] = _build_nc(nch_a, nch_b, nblk, use_bf16)
    nc = _NC_CACHE[key]

    fdt = ml_dtypes.bfloat16 if use_bf16 else np.float32
    bones = np.kron(np.eye(32), np.ones((4, 1))).astype(fdt)
    ident = np.eye(WIN, dtype=fdt)

    common = {
        "xext": xext,
        "bones": bones,
        "ident": ident,
    }
    for l in range(3):
        common[f"wl{l}"] = np.asarray(inputs[f"Wl{l}"]).astype(fdt)
        wse = np.concatenate(
            [
                np.asarray(inputs[f"Ws{l}"], np.float32),
                (np.asarray(inputs[f"bl{l}"], np.float32)
                 + np.asarray(inputs[f"bs{l}"], np.float32)).reshape(1, -1),
            ],
            axis=0,
        )
        common[f"ws{l}"] = wse.astype(fdt)

    in_maps = []
    for k in range(NCORES):
        m = dict(common)
        m.update(cores[k])
        in_maps.append(m)

    from concourse.bass_utils import run_bass_kernel_spmd

    res = run_bass_kernel_spmd(nc, in_maps, core_ids=list(range(NCORES)))
    global LAST_RESULTS
    LAST_RESULTS = res
    outs = [np.asarray(res.results[k]["out"]).reshape(-1)[:SLAB]
            for k in range(NCORES)]
    return np.concatenate(outs).reshape(N, 1).astype(np.float32)


if __name__ == "__main__":
    pass

